# revision 1
# baseline (speedup 1.0000x reference)
"""Distributed Trainium2 kernel for AttributeHypergraphModel (2x GATConv over
triples with attribute-attention entity embeddings).

Strategy (8 NeuronCores, SPMD):
  - nodes are relabeled on the host: sorted by (in-degree, A-side edge count)
    and dealt round-robin to cores, so every core's tile t has near-identical
    padded shapes (required: one SPMD graph).
  - attr/rel tables are projected once on device (matmul); the projected attr
    table is sharded + AllGathered, then copied to a per-core LOCAL DRAM
    tensor (gathering from the Shared scratchpad serializes all 8 cores on
    one HBM location). Entity-embedding attention and both GAT layers run on
    dma_gather'ed 512B rows. Gather indices are signed-int16-limited, so rows
    are split across two OVERLAPPING 32768-row windows (A=[0,32768),
    B=[NTOT-32768,NTOT)); ids in the overlap are assigned per 1024-row tile
    group to balance max column counts (~14% less padded traffic than a
    disjoint split). Gather chunks round-robin 4 SWDGE queues: descriptor
    generation (~8.5ns/desc) parallelizes ~4x across queues.
  - mm1 is fused into the entity-embedding tile loop: x@W = (he+te)@W0 +
    relmm[r], where relmm = rel_table @ W1 is precomputed host-side per
    relation and gathered with the rel row, so only one transpose + one
    matmul + a DVE add remain per tile. mm2 is fused into the edge-1 loop.
    Each AllGather starts right after its producer loop with no phase
    barrier or DRAM round-trip.
  - attentions are software-pipelined around the scalar-engine exp: the
    entity h/t pair interleaves score/weighted-sum stages within a tile,
    and the edge loops run one tile deep (tile g's scores issue before
    tile g-1's weighted sum), with double-buffered softmax scratch, so the
    vector engine never idles on the exp round-trip.
  - GAT node features travel as bf16 rows [h(128)|alpha_src|alpha_dst|pad]:
    AllGathered compact ([*,132] bf16, half the f32 volume), then expanded
    to 512B-stride rows during the localize copy so the edge gathers stay at
    the 512B/256B-multiple element size. alpha_src rides in the gathered row
    (col 128), so the edge attention needs no score mult+reduce — just a
    strided column extract fused with the alpha_dst add. The projected attr
    table is bf16 as well (halves its AllGather + gather bytes). bf16
    rounding puts overall rel err at ~5e-3 (gate is 2e-2).
  - each GAT layer: dst-partitioned softmax + weighted sum per 128-dst
    tile; -1e30 mask planes neutralize padding slots.
All index/mask planes are precomputed host-side; outputs are un-permuted on
the host.
"""

import sys

sys.path.insert(0, "/opt/trn_rl_repo")

import numpy as np

NCORE = 8
N = 50000
A = 16
NREL = 500
DE = 128
NPAD = 6272  # 49 tiles of 128 local slots per core
NTILE = NPAD // 128
NTOT = NPAD * NCORE  # 50176 global slots
SHARD = N // NCORE  # 6250 real rows per core (attr table + nodes)
SPLIT = 32768
WBOFF = NTOT - 32768  # window B = [WBOFF, NTOT); overlaps A = [0, 32768)
HD = 256  # GAT node-feature row (bf16): [h(128) | alpha_src | alpha_dst | pad]
NEGB = np.float32(-1.0e30)
NEG_SLOPE = 0.2


# ---------------------------------------------------------------- planning --


def _pack_idx(plane):
    """[128, c] int plane (slot p gets column j at gather position j*128+p)
    -> int16 SBUF index layout [128, 8*c] (16-row pattern replicated x8)."""
    p128, c = plane.shape
    assert p128 == 128
    assert plane.min(initial=0) >= 0 and plane.max(initial=0) < 32768
    vals = plane.T.reshape(-1)  # logical gather order
    cols = vals.size // 16
    arr = vals.reshape(cols, 16).T  # arr[i%16, i//16] = vals[i]
    return np.ascontiguousarray(np.tile(arr, (8, 1)).astype(np.int16))


def _column_planes(padded, k_a, total, c_a, c_b, split):
    """Split per-row id lists (A-first order in `padded`) into A/B column
    planes plus additive mask biases (-1e30 on padding)."""
    colA = np.arange(c_a)[None, :]
    mA = colA < k_a[:, None]
    pA = np.where(mA, padded[:, :c_a], 0).astype(np.int64)
    bA = np.where(mA, np.float32(0), NEGB).astype(np.float32)
    colB = np.arange(c_b)[None, :]
    mB = colB < (total - k_a)[:, None]
    gidx = np.minimum(k_a[:, None] + colB, padded.shape[1] - 1)
    pB = np.where(mB, np.take_along_axis(padded, gidx, axis=1) - split, 0)
    pB = pB.astype(np.int64)
    bB = np.where(mB, np.float32(0), NEGB).astype(np.float32)
    return pA, bA, pB, bB


def _build_family(ordered, kA, total, cA, cB, split):
    """ordered: [NCORE*NPAD, W] id lists (A ids first); returns per-tile
    cA/cB and per-core concatenated idx/mask planes."""
    nrow = ordered.shape[0]
    per_core = nrow // NCORE
    ntile = per_core // 128
    idx_a = [[] for _ in range(NCORE)]
    idx_b = [[] for _ in range(NCORE)]
    masks = [[] for _ in range(NCORE)]
    for c in range(NCORE):
        for t in range(ntile):
            r0 = c * per_core + t * 128
            pA, bA, pB, bB = _column_planes(
                ordered[r0 : r0 + 128], kA[r0 : r0 + 128], total[r0 : r0 + 128],
                int(cA[t]), int(cB[t]), split,
            )
            idx_a[c].append(_pack_idx(pA))
            idx_b[c].append(_pack_idx(pB))
            masks[c].append(np.concatenate([bA, bB], axis=1))
    return dict(
        cA=[int(x) for x in cA],
        cB=[int(x) for x in cB],
        idxA=[np.ascontiguousarray(np.concatenate(v, axis=1)) for v in idx_a],
        idxB=[np.ascontiguousarray(np.concatenate(v, axis=1)) for v in idx_b],
        mask=[np.ascontiguousarray(np.concatenate(v, axis=1)) for v in masks],
    )


def _two_window_assign(ids, row_len):
    """Assign each id to overlapping windows A=[0,32768) / B=[WBOFF,NTOT)
    balancing per-tile max counts. Returns (ordered ids A-first, kA)."""
    R, W = ids.shape
    pos = np.arange(W)[None, :]
    validm = pos < row_len[:, None]
    cat = np.where(validm, np.where(ids < WBOFF, 0,
                   np.where(ids < SPLIT, 1, 2)), 3)
    lo = (cat == 0).sum(1).astype(np.int64)
    hi = lo + (cat == 1).sum(1)
    lo3 = lo.reshape(NCORE, NTILE, 128)
    hi3 = hi.reshape(NCORE, NTILE, 128)
    L3 = row_len.reshape(NCORE, NTILE, 128)
    kA = np.empty_like(lo3)
    for t in range(NTILE):
        lo_t, hi_t, L_t = lo3[:, t], hi3[:, t], L3[:, t]
        best, bk = None, 0
        for k in range(int(L_t.max()) + 1):
            ka = np.clip(k, lo_t, hi_t)
            cost = ka.max() + (L_t - ka).max()
            if best is None or cost < best:
                best, bk = cost, k
        kA[:, t] = np.clip(bk, lo_t, hi_t)
    order = np.argsort(cat, axis=1, kind="stable")
    ordered = np.take_along_axis(ids, order, axis=1)
    return ordered, kA.reshape(R).astype(np.int64)


def _family_from_lists(ids, valid, split):
    """ids: [NCORE*NPAD, A] raw ids (already in table-slot space), valid rows
    marked; builds two-window A-first ordering then the family planes."""
    ids = np.where(ids < 0, 0, ids)
    row_len = np.where(valid, ids.shape[1], 0).astype(np.int64)
    ordered, kA = _two_window_assign(ids, row_len)
    total = row_len
    ordered = np.concatenate([ordered, np.zeros_like(ordered)], axis=1)
    kA3 = kA.reshape(NCORE, NTILE, 128)
    tot3 = total.reshape(NCORE, NTILE, 128)
    cA = np.maximum(kA3.max(axis=(0, 2)), 1)
    cB = np.maximum((tot3 - kA3).max(axis=(0, 2)), 1)
    return _build_family(ordered, kA, total, cA, cB, WBOFF)


def _remap_attr(ids):
    """raw attr id -> row in the padded AllGather'ed projection table."""
    return (ids // SHARD) * NPAD + (ids % SHARD)


def make_plan(h_attributes, t_attributes, r_idx, edge_index):
    h_attributes = np.asarray(h_attributes)
    t_attributes = np.asarray(t_attributes)
    r_idx = np.asarray(r_idx)
    edge_index = np.asarray(edge_index)

    src0 = np.concatenate([edge_index[0], np.arange(N, dtype=np.int64)])
    dst0 = np.concatenate([edge_index[1], np.arange(N, dtype=np.int64)])
    deg = np.bincount(dst0, minlength=N)

    def slots_from_order(order):
        rank = np.empty(N, np.int64)
        rank[order] = np.arange(N)
        core_of = rank % NCORE
        local_of = rank // NCORE
        return core_of * NPAD + local_of, core_of, local_of

    g0, _, _ = slots_from_order(np.argsort(deg, kind="stable"))
    kAe0 = np.bincount(dst0[g0[src0] < SPLIT], minlength=N)
    order = np.lexsort((kAe0, deg))
    gslot, core_of, local_of = slots_from_order(order)

    # ---- attr families (ids remapped into padded projection-table space)
    attrs_h = np.full((NCORE * NPAD, A), -1, np.int64)
    attrs_t = np.full((NCORE * NPAD, A), -1, np.int64)
    valid = np.zeros(NCORE * NPAD, bool)
    attrs_h[gslot] = _remap_attr(h_attributes)
    attrs_t[gslot] = _remap_attr(t_attributes)
    valid[gslot] = True
    fam_h = _family_from_lists(attrs_h, valid, SPLIT)
    fam_t = _family_from_lists(attrs_t, valid, SPLIT)

    # ---- r_idx gather planes
    r_slot = np.zeros(NCORE * NPAD, np.int64)
    r_slot[gslot] = r_idx
    r_slot = r_slot.reshape(NCORE, NPAD)
    ridx_planes = []
    for c in range(NCORE):
        cols = [_pack_idx(r_slot[c, t * 128 : (t + 1) * 128][:, None])
                for t in range(NTILE)]
        ridx_planes.append(np.ascontiguousarray(np.concatenate(cols, axis=1)))

    # ---- edge family (per-dst in-edge src slots, two-window A-first)
    sg = gslot[src0]
    dg = gslot[dst0]
    order_e = np.argsort(dg, kind="stable")
    sg_s = sg[order_e]
    dg_s = dg[order_e]
    cnt = np.bincount(dg_s, minlength=NTOT)
    starts = np.concatenate([[0], np.cumsum(cnt)[:-1]])
    pos = np.arange(len(sg_s)) - starts[dg_s]
    maxdeg = int(cnt.max())
    padded_e = np.zeros((NTOT, maxdeg + 8), np.int64)
    padded_e[dg_s, pos] = sg_s
    tot_e = cnt.astype(np.int64)
    ordered_e, kAe = _two_window_assign(padded_e, tot_e)
    kA3 = kAe.reshape(NCORE, NTILE, 128)
    tot3 = tot_e.reshape(NCORE, NTILE, 128)
    cAe = np.maximum(kA3.max(axis=(0, 2)), 1)
    cBe = np.maximum((tot3 - kA3).max(axis=(0, 2)), 1)
    need = int(cAe.max() + cBe.max())
    if ordered_e.shape[1] < need:
        ordered_e = np.concatenate(
            [ordered_e, np.zeros((NTOT, need - ordered_e.shape[1]), np.int64)],
            axis=1)
    fam_e = _build_family(ordered_e, kAe, tot_e, cAe, cBe, WBOFF)

    return dict(core_of=core_of, local_of=local_of,
                fam_h=fam_h, fam_t=fam_t, fam_e=fam_e, ridx=ridx_planes)


def make_weights(attr_table, rel_table, femb_w, femb_b,
                 gat1_w, gat1_asrc, gat1_adst, gat1_b,
                 gat2_w, gat2_asrc, gat2_adst, gat2_b):
    f32 = np.float32
    w = {}
    w["attr_tT"] = np.ascontiguousarray(np.asarray(attr_table, f32).T)
    import ml_dtypes
    w["rel_tT"] = np.ascontiguousarray(np.asarray(rel_table, f32).T)
    w["femb_wt"] = np.ascontiguousarray(np.asarray(femb_w, f32).T)
    w["femb_b_rep"] = np.ascontiguousarray(
        np.tile(np.asarray(femb_b, f32)[None, :], (128, 1)))
    for i, (gw, gas, gad, gb) in enumerate(
        [(gat1_w, gat1_asrc, gat1_adst, gat1_b),
         (gat2_w, gat2_asrc, gat2_adst, gat2_b)], start=1
    ):
        gw = np.asarray(gw, f32)
        aug = np.concatenate(
            [gw.T, (gw.T @ np.asarray(gas, f32))[:, None],
             (gw.T @ np.asarray(gad, f32))[:, None]], axis=1)
        w[f"waug{i}"] = np.ascontiguousarray(aug)  # [Din, 130]
        w[f"asrc{i}_rep"] = np.ascontiguousarray(
            np.tile(np.asarray(gas, f32)[None, :], (128, 1)))
        w[f"b{i}_rep"] = np.ascontiguousarray(
            np.tile(np.asarray(gb, f32)[None, :], (128, 1)))
    w["ident"] = np.eye(128, dtype=f32)
    # rel-side of mm1 folded per relation: re @ waug1[128:256] (pad to 132)
    relmm = np.asarray(rel_table, f32) @ w["waug1"][128:256]
    w["relmm"] = np.ascontiguousarray(
        np.concatenate([relmm, np.zeros((relmm.shape[0], 62), f32)], axis=1))
    return w


# ------------------------------------------------------- numpy device model --


def _sim_gather(table, idx_packed, num, elem):
    arr = idx_packed[:16]
    vals = arr.T.reshape(-1)[:num].astype(np.int64)
    rows = table[vals]
    return rows.reshape(num // 128, 128, elem).transpose(1, 0, 2)


def _fam_off(fam, t):
    oA = 8 * sum(fam["cA"][:t])
    oB = 8 * sum(fam["cB"][:t])
    oM = sum(fam["cA"][i] + fam["cB"][i] for i in range(t))
    return oA, oB, oM


def simulate(plan, weights, inputs):
    """Numpy mirror of the device program (validates the planner)."""
    import ml_dtypes
    f32 = np.float32
    attr_proj = (np.asarray(inputs["attr_table"], f32) @ weights["femb_wt"]
                 + weights["femb_b_rep"][0])
    attr_proj = attr_proj.astype(ml_dtypes.bfloat16).astype(f32)
    proj_pad = np.zeros((NTOT, DE), f32)
    for c in range(NCORE):
        proj_pad[c * NPAD : c * NPAD + SHARD] = \
            attr_proj[c * SHARD : (c + 1) * SHARD]
    rel_proj = (np.asarray(inputs["rel_table"], f32) @ weights["femb_wt"]
                + weights["femb_b_rep"][0])
    rel_proj = rel_proj.astype(ml_dtypes.bfloat16).astype(f32)
    rel_comb = np.concatenate(
        [rel_proj, np.asarray(inputs["rel_table"], f32)], axis=1)
    tab_A, tab_B = proj_pad[:SPLIT], proj_pad[WBOFF:]

    def softmax_gather(tabA, tabB, fam, core, t, query_fn, extra=None,
                       lrelu=False, elem=DE):
        cA, cB = fam["cA"][t], fam["cB"][t]
        oA, oB, oM = _fam_off(fam, t)
        gA = _sim_gather(tabA, fam["idxA"][core][:, oA : oA + 8 * cA],
                         128 * cA, elem)
        gB = _sim_gather(tabB, fam["idxB"][core][:, oB : oB + 8 * cB],
                         128 * cB, elem)
        mask = fam["mask"][core][:, oM : oM + cA + cB]
        G = np.concatenate([gA, gB], axis=1)
        if elem > DE:
            s = G[:, :, 128].copy()
            V = G[:, :, :DE]
        else:
            s = (G * query_fn()).sum(-1)
            V = G
        if extra is not None:
            s = s + extra
        s = s + mask
        if lrelu:
            s = np.maximum(s, NEG_SLOPE * s)
        m = s.max(axis=1, keepdims=True)
        ex = np.exp(s - m)
        return (V * ex[:, :, None]).sum(axis=1) / ex.sum(axis=1, keepdims=True)

    triple = np.zeros((NCORE, NPAD, 256), f32)
    for c in range(NCORE):
        for t in range(NTILE):
            rid = plan["ridx"][c][:16, 8 * t : 8 * t + 8].T.reshape(-1)[:128]
            rc = rel_comb[rid.astype(np.int64)]
            rp, re = rc[:, :128], rc[:, 128:]
            he = softmax_gather(tab_A, tab_B, plan["fam_h"], c, t,
                                lambda: rp[:, None, :])
            te = softmax_gather(tab_A, tab_B, plan["fam_t"], c, t,
                                lambda: rp[:, None, :])
            triple[c, t * 128 : (t + 1) * 128, :128] = he + te
            triple[c, t * 128 : (t + 1) * 128, 128:] = re

    def gat(x_all, waug, asrc_rep, b_rep, layer):
        import ml_dtypes
        h = x_all.reshape(NTOT, -1) @ waug
        rows = np.concatenate(
            [h[:, :130], np.zeros((NTOT, HD - 130), f32)], axis=1)
        rows = rows.astype(ml_dtypes.bfloat16).astype(f32)
        ad = h[:, 129]
        out = np.zeros((NCORE, NPAD, 128), f32)
        fam = plan["fam_e"]
        for c in range(NCORE):
            for g_i in range(NTILE):
                sl = slice(c * NPAD + g_i * 128, c * NPAD + (g_i + 1) * 128)
                agg = softmax_gather(
                    rows[:SPLIT], rows[WBOFF:], fam, c, g_i, None,
                    extra=ad[sl][:, None], lrelu=True, elem=HD)
                out[c, g_i * 128 : (g_i + 1) * 128] = agg + b_rep[0][None, :]
        return out

    x1 = gat(triple, weights["waug1"], weights["asrc1_rep"],
             weights["b1_rep"], 1)
    x2 = gat(x1, weights["waug2"], weights["asrc2_rep"], weights["b2_rep"], 2)
    return x2.reshape(NCORE, NPAD, 128)[plan["core_of"], plan["local_of"]]


# ------------------------------------------------------------ bass program --


def build_bass(plan):
    import copy as _copy
    import concourse.bass as bass
    import concourse.bacc as bacc
    import concourse.mybir as mb
    from contextlib import ExitStack

    F32 = mb.dt.float32
    BF16 = mb.dt.bfloat16
    I16 = mb.dt.int16
    fam_h, fam_t, fam_e = plan["fam_h"], plan["fam_t"], plan["fam_e"]

    nc = bacc.Bacc(target_bir_lowering=False, debug=True, num_swdge_queues=4)

    def par(name, shape, dt=F32, out=False):
        return nc.declare_dram_parameter(name, list(shape), dt, isOutput=out)

    attr_tT = par("attr_tT", [128, N])
    rel_tT = par("rel_tT", [128, NREL])
    relmm = par("relmm", [NREL, 192])
    femb_wt = par("femb_wt", [128, 128])
    femb_b_rep = par("femb_b_rep", [128, 128])
    waug1 = par("waug1", [256, 130])
    waug2 = par("waug2", [128, 130])
    asrc1_rep = par("asrc1_rep", [128, 128])
    asrc2_rep = par("asrc2_rep", [128, 128])
    b1_rep = par("b1_rep", [128, 128])
    b2_rep = par("b2_rep", [128, 128])
    ident = par("ident", [128, 128])
    ridx_p = par("ridx", list(plan["ridx"][0].shape), I16)
    famp = {}
    for nm, fam in (("h", fam_h), ("t", fam_t), ("e", fam_e)):
        famp[nm] = dict(
            idxA=par(f"{nm}_idxA", list(fam["idxA"][0].shape), I16),
            idxB=par(f"{nm}_idxB", list(fam["idxB"][0].shape), I16),
            mask=par(f"{nm}_mask", list(fam["mask"][0].shape)),
        )
    out_ext = par("out", [NPAD, 128], out=True)

    proj_own = nc.dram_tensor("proj_own", [NPAD, 128], BF16)
    d_attr = nc.dram_tensor("d_attr", [NTOT, 128], BF16)
    l_attr = nc.dram_tensor("l_attr", [NTOT, 128], BF16)
    HC = 132  # compact AllGather row: [h(128) | alpha_src | alpha_dst | pad2]
    d_rel = nc.dram_tensor("d_rel", [NREL, 512], BF16)
    h_own = nc.dram_tensor("h_own", [NPAD, HC], BF16)
    d_h = nc.dram_tensor("d_h", [NTOT, HC], BF16)
    l_h = nc.dram_tensor("l_h", [NTOT, HD], BF16)
    h2_own = nc.dram_tensor("h2_own", [NPAD, HC], BF16)
    d_h2 = nc.dram_tensor("d_h2", [NTOT, HC], BF16)
    l_h2 = nc.dram_tensor("l_h2", [NTOT, HD], BF16)

    cmax = {
        "hA": max(fam_h["cA"]), "hB": max(fam_h["cB"]),
        "tA": max(fam_t["cA"]), "tB": max(fam_t["cB"]),
        "eA": max(fam_e["cA"]), "eB": max(fam_e["cB"]),
    }
    cmb_max = max(cmax["hA"] + cmax["hB"], cmax["tA"] + cmax["tB"],
                  cmax["eA"] + cmax["eB"])
    wcols = max(cmax.values()) * 128

    st = ExitStack()

    def sb(name, shape, dt=F32):
        return st.enter_context(nc.sbuf_tensor(name, list(shape), dt))

    def psum(name, shape):
        return st.enter_context(nc.psum_tensor(name, list(shape), F32))

    s_fembwt = sb("s_fembwt", [128, 128])
    s_femb_b = sb("s_femb_b", [128, 128])
    s_waug1 = sb("s_waug1", [128, 130])
    s_waug2 = sb("s_waug2", [128, 130])
    s_asrc = [sb("s_asrc1", [128, 128]), sb("s_asrc2", [128, 128])]
    s_bias = [sb("s_b1", [128, 128]), sb("s_b2", [128, 128])]
    s_ident = sb("s_ident", [128, 128])
    s_ridx = sb("s_ridx", [128, 8 * NTILE], I16)
    s_ad = [sb("s_ad1", [128, NTILE]), sb("s_ad2", [128, NTILE])]
    s_at = [sb(f"s_at{i}", [128, 128]) for i in range(2)]
    s_proj = [sb(f"s_proj{i}", [128, 128]) for i in range(2)]
    s_projb = [sb(f"s_projb{i}", [128, 128], BF16) for i in range(2)]
    s_zb = sb("s_zb", [128, 128], BF16)
    s_rel = [sb(f"s_rel{i}", [128, 512], BF16) for i in range(3)]
    gelem = {k: (HD if k.startswith("e") else 128) for k in cmax}
    nbufs = {k: 3 for k in cmax}
    gbuf = {k: [sb(f"s_g{k}{i}", [128, cmax[k] * gelem[k]], BF16)
                for i in range(nbufs[k])] for k in cmax}
    ibuf = {k: [sb(f"s_i{k}{i}", [128, 8 * cmax[k]], I16)
                for i in range(nbufs[k])] for k in cmax}
    mbuf = {k: [sb(f"s_m{k}{i}", [128, cmax[k + "A"] + cmax[k + "B"]])
                for i in range(nbufs[k + "A"])] for k in ("h", "t", "e")}
    s_w1 = sb("s_w1", [128, wcols], BF16)
    s_sc = [sb(f"s_sc{i}", [128, cmb_max]) for i in range(2)]
    s_ex = [sb(f"s_ex{i}", [128, cmb_max]) for i in range(2)]
    s_red = [sb(f"s_red{i}", [128, 4]) for i in range(2)]
    s_acc = [sb(f"s_acc{i}", [128, 128]) for i in range(2)]
    s_acc2 = [sb(f"s_acc2{i}", [128, 128]) for i in range(2)]
    s_emb = [sb("s_embh", [128, 128]), sb("s_embt", [128, 128])]
    s_xT = [sb(f"s_xT{i}", [128, 256]) for i in range(2)]
    s_h = [sb(f"s_h{i}", [128, HC], BF16) for i in range(2)]
    s_o = [sb(f"s_o{i}", [128, 128]) for i in range(2)]
    s_z = sb("s_z", [128, 128])
    p_mm = [psum(f"p_mm{i}", [128, 130]) for i in range(2)]
    p_tr = [psum(f"p_tr{i}", [128, 128]) for i in range(2)]

    # ---------------- scheduling framework
    # DMA semaphores are split by purpose and tile parity so that every
    # wait covers the complete already-issued increment set on its sem
    # (race-detector-clean); compute sems (pe/act/dve/cc) update in issue
    # order and use plain cumulative counts.
    ENGS = ("gpsimd", "sync", "vector", "scalar", "tensor")
    SEMS = ("w", "p0a", "p0b", "ixa", "ixb", "ixc", "g0", "g1", "g2", "g3",
            "twa", "twb", "xa", "xb", "hwa", "hwb", "owa", "owb",
            "pe", "act", "dve", "cc", "gp", "msa", "msb", "pad", "loc")

    def gsnap(c):
        return tuple((f"g{q}", c[f"g{q}"]) for q in range(4))
    regs = {}
    ops = {e: [] for e in ENGS}
    cnt = {s: 0 for s in SEMS}
    last_wait = {e: {} for e in ENGS}

    def add(eng, emit, waits=(), inc=None):
        # same-engine pipelining can reorder element accesses: serialize
        # vector/scalar streams against themselves via their own sem.
        if eng == "vector":
            waits = list(waits) + [("dve", cnt["dve"])]
        elif eng == "scalar":
            waits = list(waits) + [("act", cnt["act"])]
        w = []
        for s_name, val in waits:
            if val <= 0 or last_wait[eng].get(s_name, -1) >= val:
                continue
            last_wait[eng][s_name] = val
            w.append((s_name, val))
        ops[eng].append((emit, tuple(w), inc))
        if inc:
            cnt[inc[0]] += inc[1]
        return dict(cnt)

    def pt(base, t):
        return base + ("a" if t % 2 == 0 else "b")

    def view_cf(buf_ap, c, f=128):  # [128, c*f] -> [128, c, f]
        return buf_ap.rearrange("p (c f) -> p c f", f=f)

    def rep_mid(vec_ap, c):      # [128, 128] -> [128, c, 128] (0-step mid)
        return vec_ap.unsqueeze(1).broadcast_to([vec_ap.shape[0], c, 128])

    def exp_inner(sc_ap, c):     # [128, c] -> [128, c, 128] (0-step inner)
        return sc_ap.unsqueeze(2).broadcast_to([sc_ap.shape[0], c, 128])

    def jview(buf_ap, c):        # [128, c*128] -> [128, 128, c] (j innermost)
        return buf_ap.rearrange("p (c f) -> p c f", f=128).transpose([0, 2, 1])

    # ---------------- phase W: constants
    for dst, srcp in ((s_fembwt, femb_wt), (s_femb_b, femb_b_rep),
                      (s_waug2, waug2), (s_asrc[0], asrc1_rep),
                      (s_asrc[1], asrc2_rep), (s_bias[0], b1_rep),
                      (s_bias[1], b2_rep), (s_ident, ident),
                      (s_ridx, ridx_p)):
        add("sync", lambda s, d=dst, so=srcp: s.dma_start(
            out=d[:, :], in_=so[:, :]), inc=("w", 16))
    add("sync", lambda s: s.dma_start(out=s_waug1[:, 0:130],
                                      in_=waug1[0:128, :]), inc=("w", 16))
    W = cnt["w"]
    for i in range(2):
        add("gpsimd", lambda g, i=i: g.memset(s_h[i][:, 130:HC], 0.0),
            inc=("gp", 1))
    GP_SH = cnt["gp"]

    # ---------------- phase 0: table projections
    def proj_rows(src_cols, n_rows, out_dst, marks, pbuf):
        ntl = (n_rows + 127) // 128
        for t in range(ntl):
            b = t % 2
            m = min(128, n_rows - t * 128)
            c0 = t * 128
            snap = add("sync", lambda s, b=b, c0=c0, m=m, sc=src_cols:
                       s.dma_start(out=s_at[b][:, 0:m],
                                   in_=sc[:, c0 : c0 + m]),
                       waits=[("pe", marks.get(("pe", b), 0))],
                       inc=(pt("p0", t), 16))
            snap = add("tensor", lambda te, b=b, m=m: te.matmul(
                p_tr[b][0:m, :], s_at[b][:, 0:m], s_fembwt[:, :],
                start=True, stop=True),
                waits=[(pt("p0", t), snap[pt("p0", t)]), ("w", W),
                       ("dve", marks.get(("dve", b), 0))],
                inc=("pe", 1))
            marks[("pe", b)] = snap["pe"]
            ms = pt("ms", t)
            snap = add("vector", lambda v, b=b, m=m, pbuf=pbuf:
                       v.tensor_tensor(
                           out=pbuf[b][0:m, :], in0=p_tr[b][0:m, :],
                           in1=s_femb_b[0:m, :], op=mb.AluOpType.add),
                       waits=[("pe", snap["pe"]), ("w", W),
                              (ms, marks.get(("ms", b), 0))],
                       inc=("dve", 1))
            marks[("dve", b)] = snap["dve"]
            snap = add("gpsimd", lambda g, b=b, c0=c0, m=m, od=out_dst,
                       pbuf=pbuf: g.dma_start(out=od(c0, m),
                                              in_=pbuf[b][0:m, :]),
                       waits=[("dve", snap["dve"])], inc=(ms, 16))
            marks[("ms", b)] = snap[ms]
        return marks

    marks = proj_rows(attr_tT, SHARD,
                      lambda c0, m: proj_own[c0 : c0 + m, :], {}, s_projb)
    snap = add("gpsimd", lambda g: g.memset(s_zb[:, :], 0.0), inc=("gp", 1))
    add("gpsimd", lambda g: g.dma_start(
        out=proj_own[SHARD:NPAD, :], in_=s_zb[0 : NPAD - SHARD, :]),
        waits=[("gp", snap["gp"])], inc=("pad", 16))
    marks = proj_rows(rel_tT, NREL,
                      lambda c0, m: d_rel[c0 : c0 + m, 0:128], marks, s_projb)
    add("gpsimd", lambda g: g.dma_start(
        out=d_rel[:, 128:512].bitcast(F32), in_=relmm[:, :]),
        inc=("pad", 16))
    MSA, MSB, GP = cnt["msa"], cnt["msb"], cnt["pad"]

    snap = add("gpsimd", lambda g: g.collective_compute(
        "AllGather", mb.AluOpType.bypass,
        replica_groups=[list(range(NCORE))],
        ins=[proj_own[:, :]], outs=[l_attr[:, :]]),
        waits=[("msa", MSA), ("msb", MSB), ("pad", GP)], inc=("cc", 1))
    cc_attr = snap["cc"]
    LOC_ATTR = 0  # l_attr is AG output directly; gathers gate on cc_attr

    # ---------------- families: offsets
    offs = {"h": [_fam_off(fam_h, t) for t in range(NTILE + 1)],
            "t": [_fam_off(fam_t, t) for t in range(NTILE + 1)],
            "e": [_fam_off(fam_e, t) for t in range(NTILE + 1)]}

    def issue_idx(nm, fam, t, b, reuse_dve, reuse_gt, ix_name=None):
        oA, oB, oM = offs[nm][t]
        cA, cB = fam["cA"][t], fam["cB"][t]
        pars = famp[nm]
        iA, iB = ibuf[nm + "A"][b], ibuf[nm + "B"][b]
        mB = mbuf[nm][b]
        ix = ix_name or pt("ix", t)
        add("sync", lambda s, iA=iA, oA=oA, cA=cA, pars=pars: s.dma_start(
            out=iA[:, 0 : 8 * cA], in_=pars["idxA"][:, oA : oA + 8 * cA]),
            waits=list(reuse_gt) + [("w", W)], inc=(ix, 16))
        add("sync", lambda s, iB=iB, oB=oB, cB=cB, pars=pars: s.dma_start(
            out=iB[:, 0 : 8 * cB], in_=pars["idxB"][:, oB : oB + 8 * cB]),
            inc=(ix, 16))
        snap = add("sync", lambda s, mB=mB, oM=oM, cc2=cA + cB, pars=pars:
                   s.dma_start(out=mB[:, 0 : cc2],
                               in_=pars["mask"][:, oM : oM + cc2]),
                   waits=[("dve", reuse_dve)], inc=(ix, 16))
        return snap

    def issue_gat(nm, fam, t, b, tabA, tabB, ix_snap, reuse_dve, extra_gw=(),
                  ix_name=None):
        cA, cB = fam["cA"][t], fam["cB"][t]
        bA, bB = gbuf[nm + "A"][b], gbuf[nm + "B"][b]
        iA, iB = ibuf[nm + "A"][b], ibuf[nm + "B"][b]
        elem = gelem[nm + "A"]
        ix = ix_name or pt("ix", t)
        gw = ([(ix, ix_snap[ix]), ("dve", reuse_dve)] + list(extra_gw))

        GCHUNK = 16  # <=2048 idx per gather; chunks round-robin the 4 queues

        def _gather(g, buf, ib, c0, c1, tab, qn):
            g.reg_mov(regs["g"], 128 * (c1 - c0))
            return g.dma_gather(
                out_ap=view_cf(buf[:, c0 * elem : c1 * elem], c1 - c0, elem),
                in_ap=tab, idxs_ap=ib[:, 8 * c0 : 8 * c1],
                num_idxs=128 * (c1 - c0), num_idxs_reg=regs["g"],
                elem_size=elem, single_packet=False, queue_num=qn)

        snap = None
        qn = issue_gat.next_q
        for buf, ib, cX, tab in ((bA, iA, cA, tabA), (bB, iB, cB, tabB)):
            for c0 in range(0, cX, GCHUNK):
                c1 = min(c0 + GCHUNK, cX)
                snap = add("gpsimd",
                           lambda g, buf=buf, ib=ib, c0=c0, c1=c1, tab=tab,
                           qn=qn: _gather(g, buf, ib, c0, c1, tab, qn),
                           waits=gw, inc=(f"g{qn}", 16))
                qn = (qn + 1) % 4
        issue_gat.next_q = qn
        return snap, cA, cB
    issue_gat.next_q = 0

    def attn_score(nm, cA, cB, b, query_ap_fn, first_waits, sidx,
                   extra_ap=None, lrelu=False):
        """Scores + softmax max + exp issue; returns the exp act count."""
        c = cA + cB
        elem = gelem[nm + "A"]
        bufs = (gbuf[nm + "A"][b], gbuf[nm + "B"][b])
        mask = mbuf[nm][b]
        sc_, ex_, rd_ = s_sc[sidx], s_ex[sidx], s_red[sidx]
        if elem > 128:
            # alpha_src rides in the gathered row at column 128: extract it
            # (fused with the per-dst alpha_dst add) instead of mult+reduce.
            for i, (cX, buf, o0) in enumerate(((cA, bufs[0], 0),
                                               (cB, bufs[1], cA))):
                col = view_cf(buf[:, 0 : cX * elem], cX, elem)[
                    :, :, 128:129].rearrange("p c f -> p (c f)")
                add("vector", lambda v, cX=cX, col=col, o0=o0, e=extra_ap:
                    v.tensor_scalar_add(sc_[:, o0 : o0 + cX], col, e),
                    waits=first_waits if i == 0 else (), inc=("dve", 1))
        else:
            for i, (cX, buf, o0) in enumerate(((cA, bufs[0], 0),
                                               (cB, bufs[1], cA))):
                q_ap = query_ap_fn(cX)
                add("vector", lambda v, cX=cX, buf=buf, q=q_ap:
                    v.tensor_tensor(
                        out=view_cf(s_w1[:, 0 : cX * 128], cX),
                        in0=view_cf(buf[:, 0 : cX * 128], cX), in1=q,
                        op=mb.AluOpType.mult),
                    waits=first_waits if i == 0 else (), inc=("dve", 1))
                add("vector", lambda v, cX=cX, o0=o0: v.tensor_reduce(
                    out=sc_[:, o0 : o0 + cX],
                    in_=view_cf(s_w1[:, 0 : cX * 128], cX),
                    axis=mb.AxisListType.X, op=mb.AluOpType.add),
                    inc=("dve", 1))
            if extra_ap is not None:
                add("vector", lambda v, e=extra_ap, c=c: v.tensor_scalar_add(
                    sc_[:, 0:c], sc_[:, 0:c], e), inc=("dve", 1))
        add("vector", lambda v, c=c, mask=mask: v.tensor_tensor(
            out=sc_[:, 0:c], in0=sc_[:, 0:c], in1=mask[:, 0:c],
            op=mb.AluOpType.add), inc=("dve", 1))
        if lrelu:
            add("vector", lambda v, c=c: v.tensor_scalar_mul(
                ex_[:, 0:c], sc_[:, 0:c], NEG_SLOPE), inc=("dve", 1))
            add("vector", lambda v, c=c: v.tensor_tensor(
                out=sc_[:, 0:c], in0=sc_[:, 0:c], in1=ex_[:, 0:c],
                op=mb.AluOpType.max), inc=("dve", 1))
        snap = add("vector", lambda v, c=c: v.tensor_reduce(
            out=rd_[:, 0:1], in_=sc_[:, 0:c], axis=mb.AxisListType.X,
            op=mb.AluOpType.max, negate=True), inc=("dve", 1))
        snap = add("scalar", lambda sc, c=c: sc.activation(
            out=ex_[:, 0:c], in_=sc_[:, 0:c],
            func=mb.ActivationFunctionType.Exp,
            bias=rd_[:, 0:1], accum_out=rd_[:, 1:2]),
            waits=[("dve", snap["dve"])], inc=("act", 1))
        return snap["act"]

    def attn_wsum(nm, cA, cB, b, sidx, exp_act):
        """Weighted sum after exp completes; fills s_acc[sidx]/s_red[sidx]."""
        elem = gelem[nm + "A"]
        bufs = (gbuf[nm + "A"][b], gbuf[nm + "B"][b])
        ex_, rd_ = s_ex[sidx], s_red[sidx]
        acc1, acc2 = s_acc[sidx], s_acc2[sidx]
        add("vector", lambda v: v.reciprocal(rd_[:, 2:3], rd_[:, 1:2]),
            waits=[("act", exp_act)], inc=("dve", 1))
        for i, (cX, buf, o0) in enumerate(((cA, bufs[0], 0),
                                           (cB, bufs[1], cA))):
            acc = acc1 if i == 0 else acc2
            hv = (view_cf(buf[:, 0 : cX * elem], cX, elem)[:, :, 0:128]
                  if elem > 128 else view_cf(buf[:, 0 : cX * 128], cX))
            add("vector", lambda v, cX=cX, hv=hv, o0=o0: v.tensor_tensor(
                out=view_cf(s_w1[:, 0 : cX * 128], cX),
                in0=hv,
                in1=exp_inner(ex_[:, o0 : o0 + cX], cX),
                op=mb.AluOpType.mult), inc=("dve", 1))
            add("vector", lambda v, cX=cX, acc=acc: v.tensor_reduce(
                out=acc[:, :], in_=jview(s_w1[:, 0 : cX * 128], cX),
                axis=mb.AxisListType.X, op=mb.AluOpType.add), inc=("dve", 1))
        snap = add("vector", lambda v, acc1=acc1, acc2=acc2: v.tensor_tensor(
            out=acc1[:, :], in0=acc1[:, :], in1=acc2[:, :],
            op=mb.AluOpType.add), inc=("dve", 1))
        return snap

    import os as _os
    _STOP = int(_os.environ.get("BUILD_STOP", "9"))
    if _STOP < 1:
        NT1 = 0
    else:
        NT1 = NTILE
    # ---------------- phase 1: entity embedding with fused mm1
    emb_dve_done, emb_gt_done = {}, {}
    mm_act, h_hw = {}, {}
    for t in range(NT1):
        b = t % 2
        bg = t % 3
        hw = pt("hw", t)
        ixn = ("ixa", "ixb", "ixc")[bg]
        reuse_d3 = emb_dve_done.get(t - 3, 0)
        reuse_d = emb_dve_done.get(t - 2, 0)
        reuse_gt = emb_gt_done.get(t - 3, ())

        def _relgather(g, t, bg):
            g.reg_mov(regs["g"], 128)
            return g.dma_gather(
                out_ap=s_rel[bg][:, :].unsqueeze(1),
                in_ap=d_rel[:, :], idxs_ap=s_ridx[:, 8 * t : 8 * t + 8],
                num_idxs=128, num_idxs_reg=regs["g"], elem_size=512)
        snap = add("gpsimd", lambda g, t=t, bg=bg: _relgather(g, t, bg),
                   waits=[("w", W), ("msa", MSA), ("msb", MSB),
                          ("pad", GP), ("dve", reuse_d3)],
                   inc=("g0", 16))
        issue_idx("h", fam_h, t, bg, reuse_d3, reuse_gt, ix_name=ixn)
        ix_snap = issue_idx("t", fam_t, t, bg, reuse_d3, reuse_gt,
                            ix_name=ixn)
        snap, cAh, cBh = issue_gat(
            "h", fam_h, t, bg, l_attr[0:SPLIT, :], l_attr[WBOFF:NTOT, :],
            ix_snap, reuse_d3, [("cc", cc_attr)], ix_name=ixn)
        snap, cAt, cBt = issue_gat(
            "t", fam_t, t, bg, l_attr[0:SPLIT, :], l_attr[WBOFF:NTOT, :],
            ix_snap, reuse_d3, [("cc", cc_attr)], ix_name=ixn)
        emb_gt_done[t] = gsnap(snap)
        gw = list(gsnap(snap))
        rp_fn = lambda cX, bg=bg: rep_mid(s_rel[bg][:, 0:128], cX)
        # interleave: both score stages first so each exp overlaps DVE work
        eh = attn_score("h", cAh, cBh, bg, rp_fn, gw, 0)
        et = attn_score("t", cAt, cBt, bg, rp_fn, (), 1)
        attn_wsum("h", cAh, cBh, bg, 0, eh)
        add("vector", lambda v: v.tensor_scalar_mul(
            s_emb[0][:, :], s_acc[0][:, :], s_red[0][:, 2:3]),
            inc=("dve", 1))
        attn_wsum("t", cAt, cBt, bg, 1, et)
        add("vector", lambda v: v.tensor_scalar_mul(
            s_emb[1][:, :], s_acc[1][:, :], s_red[1][:, 2:3]),
            inc=("dve", 1))
        snap = add("vector", lambda v, b=b: v.tensor_tensor(
            out=s_o[b][:, :], in0=s_emb[0][:, :], in1=s_emb[1][:, :],
            op=mb.AluOpType.add), inc=("dve", 1))
        # fused mm1: x@W = (he+te)@W0 + relmm[r]  -> h_own
        sd = snap["dve"]
        snap = add("tensor", lambda te, b=b: te.transpose(
            out=p_tr[b][:, :], in_=s_o[b][:, :], identity=s_ident[:, :]),
            waits=[("dve", sd), ("act", mm_act.get(t - 2, 0)), ("w", W)],
            inc=("pe", 1))
        snap = add("scalar", lambda sc, b=b: sc.activation(
            out=s_xT[b][:, 0:128], in_=p_tr[b][:, :],
            func=mb.ActivationFunctionType.Copy),
            waits=[("pe", snap["pe"])], inc=("act", 1))
        mm_act[t] = snap["act"]
        snap = add("tensor", lambda te, b=b: te.matmul(
            p_mm[b][:, :], s_xT[b][:, 0:128], s_waug1[:, 0:130],
            start=True, stop=True),
            waits=[("act", snap["act"]), ("dve", reuse_d)], inc=("pe", 1))
        rmm = s_rel[bg][:, 128:392].bitcast(F32)
        snap = add("vector", lambda v, b=b, rmm=rmm: v.tensor_tensor(
            out=s_h[b][:, 0:130], in0=p_mm[b][:, 0:130],
            in1=rmm[:, 0:130], op=mb.AluOpType.add),
            waits=[("pe", snap["pe"]), ("gp", GP_SH),
                   (hw, h_hw.get(t - 2, 0))],
            inc=("dve", 1))
        snap = add("vector", lambda v, b=b, t=t, rmm=rmm: v.tensor_tensor(
            out=s_ad[0][:, t : t + 1], in0=p_mm[b][:, 129:130],
            in1=rmm[:, 129:130], op=mb.AluOpType.add), inc=("dve", 1))
        emb_dve_done[t] = snap["dve"]
        snap = add("gpsimd", lambda g, t=t, b=b: g.dma_start(
            out=h_own[128 * t : 128 * (t + 1), :], in_=s_h[b][:, :]),
            waits=[("dve", snap["dve"])], inc=(hw, 16))
        h_hw[t] = snap[hw]
    mm1 = dict(cnt)
    PE_ENT = cnt["pe"]
    if _STOP >= 3:
        snap = add("gpsimd", lambda g: g.collective_compute(
            "AllGather", mb.AluOpType.bypass,
            replica_groups=[list(range(NCORE))],
            ins=[h_own[:, :]], outs=[d_h[:, :]]),
            waits=[("hwa", mm1["hwa"]), ("hwb", mm1["hwb"])], inc=("cc", 1))
        cc_h1 = snap["cc"]
        add("sync", lambda s: s.dma_start(out=l_h[:, 0:HC],
                                          in_=d_h[:, :]),
            waits=[("cc", cc_h1)], inc=("loc", 16))
        # pad cols must be finite (gathered but unread): refill from d_h
        snap = add("sync", lambda s: s.dma_start(out=l_h[:, HC:HD],
                                                 in_=d_h[:, 0 : HD - HC]),
                   inc=("loc", 16))
        LOC_H1 = snap["loc"]
    else:
        cc_h1 = 0
        LOC_H1 = LOC_ATTR

    # ---------------- edge phases (layer 1 fuses mm2 -> h2_own)
    # Software-pipelined one tile deep: tile g's score stage runs while tile
    # g-1's exp completes, so the DVE never idles on the Act round-trip.
    def edge_phase(layer, d_tab, out_dst, loc_need, so_guard, fuse=False):
        ed_done, ed_gt, tinfo = {}, {}, {}
        ed_ow = {-2: so_guard, -1: so_guard}
        mm2_act = {-2: mm_act.get(NTILE - 1, 0), -1: mm_act.get(NTILE - 1, 0)}
        h2_hw = {}
        bias = s_bias[layer - 1]
        asr = s_asrc[layer - 1]
        ad_col = s_ad[layer - 1]

        def tail(g_i):
            b = g_i % 2
            ow, hw = pt("ow", g_i), pt("hw", g_i)
            cA, cB, exp_act = tinfo.pop(g_i)
            attn_wsum("e", cA, cB, g_i % 3, b, exp_act)
            snap = add("vector", lambda v, b=b: v.tensor_scalar_mul(
                s_o[b][:, :], s_acc[b][:, :], s_red[b][:, 2:3]),
                waits=[ed_ow.get(g_i - 2)], inc=("dve", 1))
            snap = add("vector", lambda v, b=b, bias=bias: v.tensor_tensor(
                out=s_o[b][:, :], in0=s_o[b][:, :], in1=bias[:, :],
                op=mb.AluOpType.add), inc=("dve", 1))
            ed_done[g_i] = snap["dve"]
            if fuse:
                # fused mm2: transpose s_o -> matmul waug2 -> h2_own
                snap = add("tensor", lambda te, b=b: te.transpose(
                    out=p_tr[b][:, :], in_=s_o[b][:, :],
                    identity=s_ident[:, :]),
                    waits=[("dve", snap["dve"]),
                           ("act", mm2_act.get(g_i - 2, 0)), ("w", W)],
                    inc=("pe", 1))
                ed_ow[g_i] = ("pe", snap["pe"])
                snap = add("scalar", lambda sc, b=b: sc.activation(
                    out=s_xT[b][:, 0:128], in_=p_tr[b][:, :],
                    func=mb.ActivationFunctionType.Copy),
                    waits=[("pe", snap["pe"])], inc=("act", 1))
                snap = add("tensor", lambda te, b=b: te.matmul(
                    p_mm[b][:, :], s_xT[b][:, 0:128], s_waug2[:, 0:130],
                    start=True, stop=True),
                    waits=[("act", snap["act"])], inc=("pe", 1))
                snap = add("scalar", lambda sc, b=b: sc.activation(
                    out=s_h[b][:, 0:130], in_=p_mm[b][:, 0:130],
                    func=mb.ActivationFunctionType.Copy),
                    waits=[("pe", snap["pe"]), ("gp", GP_SH),
                           (hw, h2_hw.get(g_i - 2, 0))], inc=("act", 1))
                snap = add("scalar", lambda sc, b=b, g_i=g_i: sc.activation(
                    out=s_ad[1][:, g_i : g_i + 1], in_=p_mm[b][:, 129:130],
                    func=mb.ActivationFunctionType.Copy), inc=("act", 1))
                mm2_act[g_i] = snap["act"]
                snap = add("gpsimd", lambda g, g_i=g_i, b=b: g.dma_start(
                    out=h2_own[128 * g_i : 128 * (g_i + 1), :],
                    in_=s_h[b][:, :]),
                    waits=[("act", snap["act"])], inc=(hw, 16))
                h2_hw[g_i] = snap[hw]
            else:
                snap = add("gpsimd", lambda g, g_i=g_i, b=b, od=out_dst:
                           g.dma_start(
                               out=od[128 * g_i : 128 * (g_i + 1), :],
                               in_=s_o[b][:, :]),
                           waits=[("dve", snap["dve"])], inc=(ow, 16))
                ed_ow[g_i] = (ow, snap[ow])

        ntl = NTILE if _STOP >= 4 else 0
        for g_i in range(ntl):
            b = g_i % 2
            bg = g_i % 3
            reuse_d = ed_done.get(g_i - 3, 0)
            reuse_gt = ed_gt.get(g_i - 3, ())
            ixn = ("ixa", "ixb", "ixc")[bg]
            ix_snap = issue_idx("e", fam_e, g_i, bg, reuse_d, reuse_gt,
                                ix_name=ixn)
            snap, cA, cB = issue_gat(
                "e", fam_e, g_i, bg, d_tab[0:SPLIT, :], d_tab[WBOFF:NTOT, :],
                ix_snap, reuse_d, [("loc", loc_need)], ix_name=ixn)
            ed_gt[g_i] = gsnap(snap)
            gw = list(gsnap(snap))
            q_fn = lambda cX, asr=asr: rep_mid(asr[:, 0:128], cX)
            exp_act = attn_score("e", cA, cB, bg, q_fn, gw, b,
                                 extra_ap=ad_col[:, g_i : g_i + 1],
                                 lrelu=True)
            tinfo[g_i] = (cA, cB, exp_act)
            if g_i >= 1:
                tail(g_i - 1)
        if ntl:
            tail(ntl - 1)
        return dict(cnt)

    e1 = edge_phase(1, l_h, None, LOC_H1, ("pe", PE_ENT), fuse=True)
    if _STOP >= 5:
        mm2 = e1
        snap = add("gpsimd", lambda g: g.collective_compute(
            "AllGather", mb.AluOpType.bypass,
            replica_groups=[list(range(NCORE))],
            ins=[h2_own[:, :]], outs=[d_h2[:, :]]),
            waits=[("hwa", mm2["hwa"]), ("hwb", mm2["hwb"])], inc=("cc", 1))
        cc_h2 = snap["cc"]
        add("sync", lambda s: s.dma_start(out=l_h2[:, 0:HC],
                                          in_=d_h2[:, :]),
            waits=[("cc", cc_h2)], inc=("loc", 16))
        snap = add("sync", lambda s: s.dma_start(out=l_h2[:, HC:HD],
                                                 in_=d_h2[:, 0 : HD - HC]),
                   inc=("loc", 16))
        LOC_H2 = snap["loc"]
        if _STOP >= 6:
            edge_phase(2, l_h2, out_ext, LOC_H2, ("pe", cnt["pe"]))

    if _STOP < 9:
        snap0 = add("gpsimd", lambda g: g.dma_start(
            out=out_ext[0:128, :], in_=s_z[:, :]), inc=("pad", 16))
    final = dict(cnt)
    import os
    if os.environ.get("BASS_PRINT_SEMS"):
        print("FINAL SEM COUNTS:", final)

    # ---------------- emit
    with ExitStack() as es:
        block = es.enter_context(nc.Block())
        sems = {s_name: es.enter_context(nc.semaphore(f"sem_{s_name}"))
                for s_name in SEMS}

        def make_body(eng_name):
            def body(eng):
                if eng_name == "gpsimd":
                    regs["g"] = es.enter_context(eng.register("gnum"))
                for emit, waits, inc in ops[eng_name]:
                    for s_name, val in waits:
                        eng.wait_ge(sems[s_name], val)
                    inst = emit(eng)
                    if inc is not None and inst is not None:
                        inst.then_inc(sems[inc[0]], inc[1])
                if eng_name == "gpsimd":
                    for s_name in SEMS:
                        if s_name != "cc" and final[s_name] > 0:
                            eng.wait_ge(sems[s_name], final[s_name])
            return body

        block.gpsimd(make_body("gpsimd"))
        block.sync(make_body("sync"))
        block.vector(make_body("vector"))
        block.scalar(make_body("scalar"))
        block.tensor(make_body("tensor"))

    nc.compile()
    st.close()
    return nc


# ---------------------------------------------------------------- kernel() --

_CACHE = {}


def _prepare(inputs):
    plan = make_plan(inputs["h_attributes"], inputs["t_attributes"],
                     inputs["r_idx"], inputs["edge_index"])
    weights = make_weights(
        inputs["attr_table"], inputs["rel_table"], inputs["femb_w"],
        inputs["femb_b"], inputs["gat1_w"], inputs["gat1_asrc"],
        inputs["gat1_adst"], inputs["gat1_b"], inputs["gat2_w"],
        inputs["gat2_asrc"], inputs["gat2_adst"], inputs["gat2_b"])
    in_maps = []
    for c in range(NCORE):
        m = dict(
            attr_tT=np.ascontiguousarray(
                np.roll(weights["attr_tT"], -c * SHARD, axis=1)),
            rel_tT=weights["rel_tT"], relmm=weights["relmm"],
            femb_wt=weights["femb_wt"], femb_b_rep=weights["femb_b_rep"],
            waug1=weights["waug1"], waug2=weights["waug2"],
            asrc1_rep=weights["asrc1_rep"], asrc2_rep=weights["asrc2_rep"],
            b1_rep=weights["b1_rep"], b2_rep=weights["b2_rep"],
            ident=weights["ident"], ridx=plan["ridx"][c],
        )
        for nm in ("h", "t", "e"):
            fam = plan[f"fam_{nm}"]
            m[f"{nm}_idxA"] = fam["idxA"][c]
            m[f"{nm}_idxB"] = fam["idxB"][c]
            m[f"{nm}_mask"] = fam["mask"][c]
        in_maps.append(m)
    return plan, weights, in_maps


LAST_EXEC_NS = None


def kernel(**inputs):
    global LAST_EXEC_NS
    import os
    plan, weights, in_maps = _prepare(inputs)
    nc = build_bass(plan)
    from concourse.bass_utils import run_bass_kernel_spmd
    trace = bool(os.environ.get("BASS_KTRACE"))
    res = run_bass_kernel_spmd(nc, in_maps, list(range(NCORE)), trace=trace)
    if res.exec_time_ns is not None:
        LAST_EXEC_NS = res.exec_time_ns
    outs = np.stack([np.asarray(res.results[c]["out"]) for c in range(NCORE)])
    return np.ascontiguousarray(
        outs[plan["core_of"], plan["local_of"]].astype(np.float32))


def bench(n_iter=5, inputs=None):
    """Time repeated NEFF executions (inputs staged once; outputs donated
    fresh each iter). Returns (best_s, all_s)."""
    import time
    import jax
    import jax.numpy as jnp
    from jax.sharding import Mesh, PartitionSpec, NamedSharding
    plan, weights, in_maps = _prepare(inputs)
    nc = build_bass(plan)
    from concourse import bass2jax

    # replicate run_bass_via_pjrt, but keep the compiled callable
    import concourse.mybir as mybir
    bass2jax.install_neuronx_cc_hook()
    partition_name = (nc.partition_id_tensor.name
                      if nc.partition_id_tensor else None)
    in_names, out_names, out_avals, zero_outs = [], [], [], []
    for alloc in nc.m.functions[0].allocations:
        if not isinstance(alloc, mybir.MemoryLocationSet):
            continue
        name = alloc.memorylocations[0].name
        if alloc.kind == "ExternalInput":
            if name != partition_name:
                in_names.append(name)
        elif alloc.kind == "ExternalOutput":
            shape = tuple(alloc.tensor_shape)
            dtype = mybir.dt.np(alloc.dtype)
            out_names.append(name)
            out_avals.append(jax.core.ShapedArray(shape, dtype))
            zero_outs.append(np.zeros(shape, dtype))
    n_params = len(in_names)
    n_outs = len(out_avals)
    in_names.extend(out_names)
    if partition_name is not None:
        in_names.append(partition_name)
    donate = tuple(range(n_params, n_params + n_outs))

    def _body(*args):
        operands = list(args)
        if partition_name is not None:
            operands.append(bass2jax.partition_id_tensor())
        return tuple(bass2jax._bass_exec_p.bind(
            *operands, out_avals=tuple(out_avals), in_names=tuple(in_names),
            out_names=tuple(out_names), lowering_input_output_aliases=(),
            sim_require_finite=True, sim_require_nnan=True, nc=nc))

    from jax.experimental.shard_map import shard_map
    devices = jax.devices()[:NCORE]
    mesh = Mesh(np.asarray(devices), ("core",))
    in_specs = (PartitionSpec("core"),) * (n_params + n_outs)
    out_specs = (PartitionSpec("core"),) * len(out_names)
    fn = jax.jit(shard_map(_body, mesh=mesh, in_specs=in_specs,
                           out_specs=out_specs, check_rep=False),
                 donate_argnums=donate, keep_unused=True)
    if nc.dbg_addr is not None:
        in_maps = [{**m, nc.dbg_addr.name: np.zeros((1, 2), np.uint32)}
                   for m in in_maps]
    per_core = [[np.asarray(m[k]) for k in in_names[:n_params]]
                for m in in_maps]
    sh = NamedSharding(mesh, PartitionSpec("core"))
    concat_in = [jax.device_put(
        np.concatenate([per_core[c][i] for c in range(NCORE)], axis=0), sh)
        for i in range(n_params)]
    zglobal = [np.zeros((NCORE * z.shape[0], *z.shape[1:]), z.dtype)
               for z in zero_outs]
    times = []
    for it in range(n_iter):
        zs = [jax.device_put(z, sh) for z in zglobal]
        for z in zs:
            z.block_until_ready()
        t0 = time.perf_counter()
        outs = fn(*concat_in, *zs)
        for o in outs:
            o.block_until_ready()
        times.append(time.perf_counter() - t0)
    return min(times), times



# revision 51
# speedup vs baseline: 2.0893x; 2.0893x over previous
"""Distributed Trainium2 kernel for AttributeHypergraphModel (2x GATConv over
triples with attribute-attention entity embeddings).

Strategy (8 NeuronCores, SPMD):
  - nodes are relabeled on the host: sorted by (in-degree, A-side edge count)
    and dealt round-robin to cores; global slots are CHUNK-MAJOR (7 chunks of
    7 tiles) so AllGather chunks land contiguously.
  - the attr/rel tables are projected REPLICATED on every core (391 tiles of
    [128x128] matmul with the femb bias folded in via a rank-1 accumulate
    matmul) straight into local DRAM l_attr — no AllGather for the table.
  - entity-embedding attention and both GAT layers run on dma_gather'ed rows
    (256B attr rows, 512B GAT rows, 768B rel rows). int16 gather indices
    cover the tables via two overlapping 32768-row windows; ids assigned
    per 1024-row tile group to balance the padded column counts. A and B
    windows gather into ONE contiguous SBUF buffer so every DVE op spans
    the merged column range.
  - DVE ops are all-bf16 with packed innermost access so they hit the 2x
    DVE mode: bf16 queries from the gathered rel row, bf16 masks, bf16
    exp output written directly in PAIRED layout ([p,c,2]) by the scalar
    engine so the weighted-sum multiply gets a packed stride-[1,2]
    broadcast operand.
  - the attention weighted sum is reduced on the TENSOR engine: per column
    slab k, an accumulating matmul with stationary 2*I sums the alpha-scaled
    rows into PSUM (the 2x compensates the doubled softmax denominator from
    the paired exp accumulation). The scalar engine then copies PSUM->SBUF
    with a per-partition 1/denom scale.
  - mm1 is fused into the entity tile loop (x@W = (he+te)@W0 + relmm[r],
    relmm precomputed host-side and riding in the gathered rel row), mm2
    into the edge-1 loop; both matmuls are bf16.
  - node-feature AllGathers are CHUNKED (7 chunks of 896 rows, compact
    [*,132]-bf16 payload) and issued from the sync queue as their producer
    tiles complete, so all but the last chunk overlap compute. Each chunk
    is localized (132->256-elem-stride rows) right after it lands; edge
    gathers then read 512B rows from local DRAM.
All index/mask planes are precomputed host-side; outputs are un-permuted on
the host.
"""

import sys

sys.path.insert(0, "/opt/trn_rl_repo")

import numpy as np

NCORE = 8
N = 50000
A = 16
NREL = 500
DE = 128
NPAD = 6272  # 49 tiles of 128 local slots per core
NTILE = NPAD // 128
# AllGather chunk sizes (tiles): small enough that each collective's
# gpsimd-blocking time (~35-42us) hides under the 5-tile gather prefetch,
# and every collective's in/out APs stay contiguous (HW requirement)
CH_TILES = [4] * 10 + [3] * 3
NCHUNK = len(CH_TILES)
CH_START = np.concatenate([[0], np.cumsum(CH_TILES)[:-1]]).astype(int)
CH_LAST = (CH_START + np.array(CH_TILES) - 1).tolist()
CH_START = CH_START.tolist()
CH_ROWS = [t * 128 for t in CH_TILES]
CH_GSTART = np.concatenate(
    [[0], np.cumsum([8 * r for r in CH_ROWS])[:-1]]).astype(int).tolist()
CHUNK_OF = [k for k, t in enumerate(CH_TILES) for _ in range(t)]
NTOT = NPAD * NCORE   # 50176 global slots
SPLIT = 32768
WBOFF_E = NTOT - 32768  # edge window B = [17408, NTOT)
WBOFF_A = N - 32768     # attr window B = [17232, N)
NATTR_PAD = 50048       # 391 tiles of projected attr rows
HD = 256  # GAT node-feature row (bf16): [h(128) | alpha_src | alpha_dst | .]
HC = 132  # compact AllGather row
RELW = 384  # d_rel row elems (bf16): [rp(128) | relmm(130) | pad]
NEGB = np.float32(-1.0e30)
NEG_SLOPE = 0.2


# ---------------------------------------------------------------- planning --


def _pack_idx(plane):
    """[128, c] int plane (slot p gets column j at gather position j*128+p)
    -> int16 SBUF index layout [128, 8*c] (16-row pattern replicated x8)."""
    p128, c = plane.shape
    assert p128 == 128
    assert plane.min(initial=0) >= 0 and plane.max(initial=0) < 32768
    vals = plane.T.reshape(-1)  # logical gather order
    cols = vals.size // 16
    arr = vals.reshape(cols, 16).T  # arr[i%16, i//16] = vals[i]
    return np.ascontiguousarray(np.tile(arr, (8, 1)).astype(np.int16))


def _column_planes(padded, k_a, total, c_a, c_b, wboff):
    """Split per-row id lists (A-first order in `padded`) into A/B column
    planes plus additive mask biases (-1e30 on padding)."""
    colA = np.arange(c_a)[None, :]
    mA = colA < k_a[:, None]
    pA = np.where(mA, padded[:, :c_a], 0).astype(np.int64)
    bA = np.where(mA, np.float32(0), NEGB).astype(np.float32)
    colB = np.arange(c_b)[None, :]
    mB = colB < (total - k_a)[:, None]
    gidx = np.minimum(k_a[:, None] + colB, padded.shape[1] - 1)
    pB = np.where(mB, np.take_along_axis(padded, gidx, axis=1) - wboff, 0)
    pB = pB.astype(np.int64)
    bB = np.where(mB, np.float32(0), NEGB).astype(np.float32)
    return pA, bA, pB, bB


def _slot_rows(c, t):
    """Global-slot row range of (core c, tile t) in chunk-major layout."""
    k = CHUNK_OF[t]
    return CH_GSTART[k] + c * CH_ROWS[k] + (t - CH_START[k]) * 128


def _build_family(ordered, kA, total, cA, cB, wboff):
    """ordered: [NTOT, W] id lists (A ids first) indexed by GLOBAL SLOT;
    returns per-tile cA/cB and per-core concatenated merged idx/mask
    planes (A then B)."""
    import ml_dtypes
    idx = [[] for _ in range(NCORE)]
    masks = [[] for _ in range(NCORE)]
    for c in range(NCORE):
        for t in range(NTILE):
            r0 = _slot_rows(c, t)
            pA, bA, pB, bB = _column_planes(
                ordered[r0 : r0 + 128], kA[r0 : r0 + 128], total[r0 : r0 + 128],
                int(cA[t]), int(cB[t]), wboff,
            )
            idx[c].append(np.concatenate(
                [_pack_idx(pA), _pack_idx(pB)], axis=1))
            masks[c].append(np.concatenate([bA, bB], axis=1))
    return dict(
        cA=[int(x) for x in cA],
        cB=[int(x) for x in cB],
        idx=[np.ascontiguousarray(np.concatenate(v, axis=1)) for v in idx],
        mask=[np.ascontiguousarray(
            np.concatenate(v, axis=1).astype(ml_dtypes.bfloat16))
            for v in masks],
    )


def _two_window_assign(ids, row_len, wboff):
    """Assign each id to overlapping windows A=[0,32768) / B=[wboff,..)
    balancing per-tile max counts. Returns (ordered ids A-first, kA).
    Rows are indexed by GLOBAL SLOT (chunk-major)."""
    R, W = ids.shape
    pos = np.arange(W)[None, :]
    validm = pos < row_len[:, None]
    cat = np.where(validm, np.where(ids < wboff, 0,
                   np.where(ids < SPLIT, 1, 2)), 3)
    lo = (cat == 0).sum(1).astype(np.int64)
    hi = lo + (cat == 1).sum(1)

    # group rows by tile id (over all cores)
    tile_of = np.empty(R, np.int64)
    for c in range(NCORE):
        for t in range(NTILE):
            r0 = _slot_rows(c, t)
            tile_of[r0 : r0 + 128] = t
    kA = np.empty(R, np.int64)
    for t in range(NTILE):
        sel = tile_of == t
        lo_t, hi_t, L_t = lo[sel], hi[sel], row_len[sel]
        best, bk = None, 0
        for k in range(int(L_t.max()) + 1):
            ka = np.clip(k, lo_t, hi_t)
            cost = ka.max() + (L_t - ka).max()
            if best is None or cost < best:
                best, bk = cost, k
        kA[sel] = np.clip(bk, lo_t, hi_t)
    order = np.argsort(cat, axis=1, kind="stable")
    ordered = np.take_along_axis(ids, order, axis=1)
    return ordered, kA


def _tile_counts(kA, total):
    tile_of = np.empty(NTOT, np.int64)
    for c in range(NCORE):
        for t in range(NTILE):
            r0 = _slot_rows(c, t)
            tile_of[r0 : r0 + 128] = t
    cA = np.zeros(NTILE, np.int64)
    cB = np.zeros(NTILE, np.int64)
    for t in range(NTILE):
        sel = tile_of == t
        cA[t] = max(int(kA[sel].max()), 1)
        cB[t] = max(int((total[sel] - kA[sel]).max()), 1)
    return cA, cB


def _family_from_lists(ids, valid, wboff):
    """ids: [NTOT, A] raw ids, indexed by global slot; valid rows marked."""
    ids = np.where(ids < 0, 0, ids)
    row_len = np.where(valid, ids.shape[1], 0).astype(np.int64)
    ordered, kA = _two_window_assign(ids, row_len, wboff)
    cA, cB = _tile_counts(kA, row_len)
    ordered = np.concatenate([ordered, np.zeros_like(ordered)], axis=1)
    return _build_family(ordered, kA, row_len, cA, cB, wboff)


def attr_perm(h_attributes, t_attributes):
    """Table positions for attr ids: the most-used attrs go to the
    window-overlap (flex) region [WBOFF_A, SPLIT) so the per-tile A/B
    column balancing has maximum freedom; the rest alternate into the
    A-only / B-only regions by use rank."""
    counts = (np.bincount(np.asarray(h_attributes).ravel(), minlength=N)
              + np.bincount(np.asarray(t_attributes).ravel(), minlength=N))
    order = np.argsort(-counts, kind="stable")
    nflex = SPLIT - WBOFF_A
    newpos = np.empty(N, np.int64)
    newpos[order[:nflex]] = WBOFF_A + np.arange(nflex)
    rest = order[nflex:]
    newpos[rest[0::2]] = np.arange(len(rest[0::2]))
    newpos[rest[1::2]] = SPLIT + np.arange(len(rest[1::2]))
    return newpos


def make_plan(h_attributes, t_attributes, r_idx, edge_index, newpos):
    h_attributes = newpos[np.asarray(h_attributes)]
    t_attributes = newpos[np.asarray(t_attributes)]
    r_idx = np.asarray(r_idx)
    edge_index = np.asarray(edge_index)

    src0 = np.concatenate([edge_index[0], np.arange(N, dtype=np.int64)])
    dst0 = np.concatenate([edge_index[1], np.arange(N, dtype=np.int64)])
    deg = np.bincount(dst0, minlength=N)

    k_l = np.array(CHUNK_OF)[np.arange(NPAD) // 128]
    g0_l = np.array(CH_GSTART)[k_l]
    rows_l = np.array(CH_ROWS)[k_l]
    off_l = np.arange(NPAD) - np.array(CH_START)[k_l] * 128

    def slots_from_order(order):
        rank = np.empty(N, np.int64)
        rank[order] = np.arange(N)
        core_of = rank % NCORE
        local_of = rank // NCORE
        gslot = g0_l[local_of] + core_of * rows_l[local_of] \
            + off_l[local_of]
        return gslot, core_of, local_of

    g0, _, _ = slots_from_order(np.argsort(deg, kind="stable"))
    kAe0 = np.bincount(dst0[g0[src0] < SPLIT], minlength=N)
    order = np.lexsort((kAe0, deg))
    gslot, core_of, local_of = slots_from_order(order)

    # ---- attr families (raw attr-table ids, no remap)
    attrs_h = np.full((NTOT, A), -1, np.int64)
    attrs_t = np.full((NTOT, A), -1, np.int64)
    valid = np.zeros(NTOT, bool)
    attrs_h[gslot] = h_attributes
    attrs_t[gslot] = t_attributes
    valid[gslot] = True
    fam_h = _family_from_lists(attrs_h, valid, WBOFF_A)
    fam_t = _family_from_lists(attrs_t, valid, WBOFF_A)

    # ---- r_idx gather planes (per core, [128, 8*NTILE])
    r_slot = np.zeros(NTOT, np.int64)
    r_slot[gslot] = r_idx
    ridx_planes = []
    for c in range(NCORE):
        cols = []
        for t in range(NTILE):
            r0 = _slot_rows(c, t)
            cols.append(_pack_idx(r_slot[r0 : r0 + 128][:, None]))
        ridx_planes.append(np.ascontiguousarray(np.concatenate(cols, axis=1)))

    # ---- edge family (per-dst in-edge src slots, two-window A-first)
    sg = gslot[src0]
    dg = gslot[dst0]
    order_e = np.argsort(dg, kind="stable")
    sg_s = sg[order_e]
    dg_s = dg[order_e]
    cnt = np.bincount(dg_s, minlength=NTOT)
    starts = np.concatenate([[0], np.cumsum(cnt)[:-1]])
    pos = np.arange(len(sg_s)) - starts[dg_s]
    maxdeg = int(cnt.max())
    padded_e = np.zeros((NTOT, maxdeg + 8), np.int64)
    padded_e[dg_s, pos] = sg_s
    tot_e = cnt.astype(np.int64)
    ordered_e, kAe = _two_window_assign(padded_e, tot_e, WBOFF_E)
    cAe, cBe = _tile_counts(kAe, tot_e)
    need = int(cAe.max() + cBe.max())
    if ordered_e.shape[1] < need:
        ordered_e = np.concatenate(
            [ordered_e, np.zeros((NTOT, need - ordered_e.shape[1]), np.int64)],
            axis=1)
    fam_e = _build_family(ordered_e, kAe, tot_e, cAe, cBe, WBOFF_E)

    return dict(core_of=core_of, local_of=local_of,
                fam_h=fam_h, fam_t=fam_t, fam_e=fam_e, ridx=ridx_planes)


def make_weights(attr_table, rel_table, femb_w, femb_b,
                 gat1_w, gat1_asrc, gat1_adst, gat1_b,
                 gat2_w, gat2_asrc, gat2_adst, gat2_b, newpos):
    import ml_dtypes
    f32 = np.float32
    bf16 = ml_dtypes.bfloat16
    w = {}
    at = np.asarray(attr_table, f32)
    atT = at.T
    atp = np.empty_like(atT)
    atp[:, newpos] = atT
    w["attr_tT"] = np.ascontiguousarray(atp)
    w["rel_tT"] = np.ascontiguousarray(np.asarray(rel_table, f32).T)
    w["femb_wt"] = np.ascontiguousarray(np.asarray(femb_w, f32).T)
    w["femb_b_row"] = np.ascontiguousarray(
        np.asarray(femb_b, f32)[None, :].astype(bf16))
    for i, (gw, gas, gad, gb) in enumerate(
        [(gat1_w, gat1_asrc, gat1_adst, gat1_b),
         (gat2_w, gat2_asrc, gat2_adst, gat2_b)], start=1
    ):
        gw = np.asarray(gw, f32)
        aug = np.concatenate(
            [gw.T, (gw.T @ np.asarray(gas, f32))[:, None],
             (gw.T @ np.asarray(gad, f32))[:, None]], axis=1)
        w[f"waug{i}"] = np.ascontiguousarray(
            aug[:128].astype(bf16) if i == 1 else aug.astype(bf16))
        if i == 1:
            aug1_rel = aug[128:256]  # rel half folds into relmm
        w[f"b{i}_rep"] = np.ascontiguousarray(np.tile(
            np.asarray(gb, f32)[None, :], (128, 1)).astype(
                bf16 if i == 1 else f32))
    w["ident2"] = np.ascontiguousarray((2.0 * np.eye(128)).astype(bf16))
    w["ident1"] = np.ascontiguousarray(np.eye(128, dtype=f32))
    relmm = np.asarray(rel_table, f32) @ aug1_rel  # [NREL, 130]
    w["relmm"] = np.ascontiguousarray(np.concatenate(
        [relmm, np.zeros((NREL, 2), f32)], axis=1).astype(bf16))
    return w


# ------------------------------------------------------------ bass program --


def build_bass(plan):
    import concourse.bass as bass
    import concourse.bacc as bacc
    import concourse.mybir as mb
    from contextlib import ExitStack

    F32 = mb.dt.float32
    BF16 = mb.dt.bfloat16
    I16 = mb.dt.int16
    fam_h, fam_t, fam_e = plan["fam_h"], plan["fam_t"], plan["fam_e"]

    nc = bacc.Bacc(target_bir_lowering=False, debug=True, num_swdge_queues=4)

    def par(name, shape, dt=F32, out=False):
        return nc.declare_dram_parameter(name, list(shape), dt, isOutput=out)

    attr_tT = par("attr_tT", [128, N])
    rel_tT = par("rel_tT", [128, NREL])
    relmm = par("relmm", [NREL, HC], BF16)
    femb_wt = par("femb_wt", [128, 128])
    femb_b_row = par("femb_b_row", [1, 128], BF16)
    waug1 = par("waug1", [128, 130], BF16)
    waug2 = par("waug2", [128, 130], BF16)
    b1_rep = par("b1_rep", [128, 128], BF16)
    b2_rep = par("b2_rep", [128, 128], F32)
    ident2 = par("ident2", [128, 128], BF16)
    ident1 = par("ident1", [128, 128])
    ridx_p = par("ridx", list(plan["ridx"][0].shape), I16)
    famp = {}
    for nm, fam in (("h", fam_h), ("t", fam_t), ("e", fam_e)):
        famp[nm] = dict(
            idx=par(f"{nm}_idx", list(fam["idx"][0].shape), I16),
            mask=par(f"{nm}_mask", list(fam["mask"][0].shape), BF16),
        )
    out_ext = par("out", [NPAD, 128], out=True)

    l_attr = nc.dram_tensor("l_attr", [NATTR_PAD, 128], BF16)
    d_rel = nc.dram_tensor("d_rel", [512, RELW], BF16)
    h_own = nc.dram_tensor("h_own", [NPAD, HC], BF16)
    d_h = nc.dram_tensor("d_h", [NTOT, HC], BF16)
    l_h = nc.dram_tensor("l_h", [NTOT, HD], BF16)
    h2_own = nc.dram_tensor("h2_own", [NPAD, HC], BF16)
    d_h2 = nc.dram_tensor("d_h2", [NTOT, HC], BF16)
    l_h2 = nc.dram_tensor("l_h2", [NTOT, HD], BF16)

    cm = {
        "h": max(fam_h["cA"][t] + fam_h["cB"][t] for t in range(NTILE)),
        "t": max(fam_t["cA"][t] + fam_t["cB"][t] for t in range(NTILE)),
        "e": max(fam_e["cA"][t] + fam_e["cB"][t] for t in range(NTILE)),
    }
    cmb_max = max(cm.values())
    gelem = {"h": DE, "t": DE, "e": HD}

    st = ExitStack()

    def sb(name, shape, dt=F32):
        return st.enter_context(nc.sbuf_tensor(name, list(shape), dt))

    def psum(name, shape):
        return st.enter_context(nc.psum_tensor(name, list(shape), F32))

    s_fembwt = sb("s_fembwt", [128, 128])
    s_brow = sb("s_brow", [1, 128], BF16)
    s_ones = sb("s_ones", [1, 128], BF16)
    s_waug1 = sb("s_waug1", [128, 130], BF16)
    s_waug2 = sb("s_waug2", [128, 130], BF16)
    s_bias1 = sb("s_bias1", [128, 128], BF16)
    s_bias2 = sb("s_bias2", [128, 128], F32)
    s_id2 = sb("s_id2", [128, 128], BF16)
    s_id1 = sb("s_id1", [128, 128])
    s_ridx = sb("s_ridx", [128, 8 * NTILE], I16)
    s_ad = [sb("s_ad1", [128, NTILE]), sb("s_ad2", [128, NTILE])]
    s_at = [sb(f"s_at{i}", [128, 128 * 4]) for i in range(2)]
    s_projb = [sb(f"s_projb{i}", [128, 128 * 4], BF16)
               for i in range(2)]
    NB = {"h": 5, "t": 5, "e": 3}  # prefetch depth per family
    s_rel = [sb(f"s_rel{i}", [128, RELW], BF16) for i in range(5)]
    gbuf = {k: [sb(f"s_g{k}{i}", [128, cm[k] * gelem[k]], BF16)
                for i in range(NB[k])] for k in cm}
    ibuf = {k: [sb(f"s_i{k}{i}", [128, 8 * cm[k]], I16)
                for i in range(NB[k])] for k in cm}
    mbuf = {k: [sb(f"s_m{k}{i}", [128, cm[k]], BF16)
                for i in range(NB[k])] for k in cm}
    s_w1 = [sb(f"s_w1{i}", [128, cmb_max * 128], BF16) for i in range(2)]
    s_sc = [sb(f"s_sc{i}", [128, cmb_max]) for i in range(2)]
    s_lr = [sb(f"s_lr{i}", [128, cmb_max]) for i in range(2)]
    s_ep = [sb(f"s_ep{i}", [128, 2 * cmb_max], BF16) for i in range(2)]
    s_red = [sb(f"s_red{i}", [128, 4]) for i in range(2)]
    s_emb = [sb("s_embh", [128, 128], BF16), sb("s_embt", [128, 128], BF16)]
    s_o = [sb(f"s_o{i}", [128, 128]) for i in range(2)]
    s_of = [sb(f"s_of{i}", [128, 128]) for i in range(2)]
    s_xT = [sb(f"s_xT{i}", [128, 128], BF16) for i in range(2)]
    s_h = [sb(f"s_h{i}", [128, HC], BF16) for i in range(2)]
    p_proj = [psum(f"p_proj{i}", [128, 128 * 4]) for i in range(2)]
    p_mm = [psum(f"p_mm{i}", [128, 130]) for i in range(2)]
    p_tr = [psum(f"p_tr{i}", [128, 128]) for i in range(2)]
    p_acc = [psum(f"p_acc{i}", [128, 128]) for i in range(2)]

    # ---------------- scheduling framework (see baseline): DMA semaphores
    # split by purpose/parity; compute sems use cumulative issue counts.
    ENGS = ("gpsimd", "sync", "vector", "scalar", "tensor")
    # gather sems are split by SWDGE queue AND 3-deep rotation group so a
    # consumer's wait always covers the complete issued increment set on
    # its sem even with the gather stream running 3 tiles ahead
    # (DMA-completion increments are unordered for the race detector;
    # engine increments -- pe/act/dve/ckf -- are ordered, prefix waits ok).
    SEMS = (("w", "p0a", "p0b", "ix0", "ix1", "ix2", "ix3", "ix4",
             "hwa", "hwb", "owa", "owb", "ckf", "ckf2",
             "c1a", "c1b", "c2a", "c2b", "loc1", "loc2",
             "pe", "act", "dve", "gp", "msa", "msb", "pad")
            + tuple(f"g{q}{r}" for q in range(4) for r in range(5)))

    def gsnap(c, r):
        return tuple((f"g{q}{r}", c[f"g{q}{r}"]) for q in range(4))
    regs = {}
    ops = {e: [] for e in ENGS}
    cnt = {s: 0 for s in SEMS}
    last_wait = {e: {} for e in ENGS}

    def add(eng, emit, waits=(), inc=None):
        if eng == "vector":
            waits = list(waits) + [("dve", cnt["dve"])]
        elif eng == "scalar":
            waits = list(waits) + [("act", cnt["act"])]
        w = []
        for s_name, val in waits:
            if val <= 0 or last_wait[eng].get(s_name, -1) >= val:
                continue
            last_wait[eng][s_name] = val
            w.append((s_name, val))
        ops[eng].append((emit, tuple(w), inc))
        if inc:
            cnt[inc[0]] += inc[1]
        return dict(cnt)

    def pt(base, t):
        return base + ("a" if t % 2 == 0 else "b")

    def view_cf(buf_ap, c, f=128):  # [128, c*f] -> [128, c, f]
        return buf_ap.rearrange("p (c f) -> p c f", f=f)

    def view4(buf_ap, c):  # [128, c*128] -> [128, c, 64, 2]
        return buf_ap.rearrange("p (c f two) -> p c f two", f=64, two=2)

    def rep_mid(vec_ap, c):      # [128, 128] -> [128, c, 128] (0-step mid)
        return vec_ap.unsqueeze(1).broadcast_to([vec_ap.shape[0], c, 128])

    def ep_view(ep, c):          # [128, 2c] -> [128, c, 2]
        return ep[:, 0 : 2 * c].rearrange("p (c two) -> p c two", two=2)

    def ep_bcast(ep, c):         # [128, 2c] -> [128, c, 64, 2] (0-step f)
        return (ep[:, 0 : 2 * c].rearrange("p (c two) -> p c two", two=2)
                .unsqueeze(2).broadcast_to([128, c, 64, 2]))

    # ---------------- phase W: constants
    for dst, srcp in ((s_fembwt, femb_wt), (s_brow, femb_b_row),
                      (s_waug1, waug1), (s_waug2, waug2),
                      (s_bias1, b1_rep), (s_bias2, b2_rep),
                      (s_id2, ident2), (s_id1, ident1), (s_ridx, ridx_p)):
        add("sync", lambda s, d=dst, so=srcp: s.dma_start(
            out=d[:, :], in_=so[:, :]), inc=("w", 16))
    W = cnt["w"]
    add("gpsimd", lambda g: g.memset(s_ones[:, :], 1.0), inc=("gp", 1))
    for i in range(2):
        add("gpsimd", lambda g, i=i: g.memset(s_h[i][:, 130:HC], 0.0),
            inc=("gp", 1))
    GP_SH = cnt["gp"]

    # ---------------- phase 0: replicated table projections (no AllGather)
    # batched PB tiles per DMA / PSUM group: the per-instruction DMA issue
    # overheads (~0.5-1.1us each) dominate this phase at 1-tile granularity
    PB = 4

    def proj_rows(src_cols, n_rows, out_dst, marks):
        # groups are padded to whole 128-row tiles; pad lanes compute on
        # garbage SBUF/psum data and land in scratch rows never gathered
        ngr = (n_rows + 128 * PB - 1) // (128 * PB)
        for g in range(ngr):
            b = g % 2
            r0 = g * 128 * PB
            m = min(128 * PB, n_rows - r0)
            ntl = (m + 127) // 128
            mp = ntl * 128
            snap = add("sync", lambda s, b=b, r0=r0, m=m, sc=src_cols:
                       s.dma_start(out=s_at[b][:, 0:m],
                                   in_=sc[:, r0 : r0 + m]),
                       waits=[("pe", marks.get(("pe", b), 0))],
                       inc=(pt("p0", g), 16))
            pw = [(pt("p0", g), snap[pt("p0", g)]), ("w", W), ("gp", GP_SH),
                  ("act", marks.get(("act", b), 0))]
            for j in range(ntl):
                add("tensor", lambda te, b=b, j=j: te.matmul(
                    p_proj[b][:, 128 * j : 128 * j + 128],
                    s_at[b][:, 128 * j : 128 * j + 128], s_fembwt[:, :],
                    start=True, stop=False),
                    waits=pw if j == 0 else (), inc=("pe", 1))
                snap = add("tensor", lambda te, b=b, j=j: te.matmul(
                    p_proj[b][:, 128 * j : 128 * j + 128],
                    s_ones[0:1, :], s_brow[0:1, :],
                    start=False, stop=True), inc=("pe", 1))
            marks[("pe", b)] = snap["pe"]
            ms = pt("ms", g)
            snap = add("scalar", lambda sc, b=b, mp=mp: sc.activation(
                out=s_projb[b][:, 0:mp], in_=p_proj[b][:, 0:mp],
                func=mb.ActivationFunctionType.Copy),
                waits=[("pe", snap["pe"]),
                       (ms, marks.get(("ms", b), 0))],
                inc=("act", 1))
            marks[("act", b)] = snap["act"]
            snap = add("gpsimd", lambda g_, b=b, r0=r0, mp=mp, od=out_dst:
                       g_.dma_start(out=od(r0, mp),
                                    in_=(s_projb[b][:, 0:mp]
                                         .rearrange("p (j f) -> p j f",
                                                    f=128))),
                       waits=[("act", snap["act"])], inc=(ms, 16))
            marks[("ms", b)] = snap[ms]
        return marks

    def attr_dst(r0, m):
        return (l_attr[r0 : r0 + m, :]
                .rearrange("(j p) f -> p j f", p=128))

    def rel_dst(r0, m):
        return (d_rel[r0 : r0 + m, 0:128]
                .rearrange("(j p) f -> p j f", p=128))

    marks = proj_rows(attr_tT, N, attr_dst, {})
    marks = proj_rows(rel_tT, NREL, rel_dst, marks)
    add("gpsimd", lambda g: g.dma_start(
        out=d_rel[0:NREL, 128 : 128 + HC], in_=relmm[:, :]),
        inc=("pad", 16))
    MSA, MSB, GP = cnt["msa"], cnt["msb"], cnt["pad"]

    # ---------------- families: offsets into merged planes
    def fam_offs(fam):
        offs = []
        oI = oM = 0
        for t in range(NTILE):
            offs.append((oI, oM))
            oI += 8 * (fam["cA"][t] + fam["cB"][t])
            oM += fam["cA"][t] + fam["cB"][t]
        return offs
    offs = {"h": fam_offs(fam_h), "t": fam_offs(fam_t), "e": fam_offs(fam_e)}

    def issue_idx(nm, fam, t, bg, reuse_dve, reuse_gt, ixn):
        oI, oM = offs[nm][t]
        c = fam["cA"][t] + fam["cB"][t]
        pars = famp[nm]
        add("sync", lambda s, iB=ibuf[nm][bg], oI=oI, c=c, pars=pars:
            s.dma_start(out=iB[:, 0 : 8 * c],
                        in_=pars["idx"][:, oI : oI + 8 * c]),
            waits=list(reuse_gt) + [("w", W)], inc=(ixn, 16))
        snap = add("sync", lambda s, mB=mbuf[nm][bg], oM=oM, c=c, pars=pars:
                   s.dma_start(out=mB[:, 0:c],
                               in_=pars["mask"][:, oM : oM + c]),
                   waits=[("dve", reuse_dve)], inc=(ixn, 16))
        return snap

    def issue_gat(nm, fam, t, bg, tabA, tabB, ix_snap, reuse_dve,
                  extra_gw=(), ixn=None):
        cA, cB = fam["cA"][t], fam["cB"][t]
        buf = gbuf[nm][bg]
        ib = ibuf[nm][bg]
        elem = gelem[nm]
        gw = ([(ixn, ix_snap[ixn]), ("dve", reuse_dve)] + list(extra_gw))

        GCHUNK = 16  # <=2048 idx per gather; chunks round-robin the 4 queues

        def _gather(g, c0, c1, i0, tab, qn):
            g.reg_mov(regs["g"], 128 * (c1 - c0))
            return g.dma_gather(
                out_ap=view_cf(buf[:, c0 * elem : c1 * elem], c1 - c0, elem),
                in_ap=tab, idxs_ap=ib[:, 8 * i0 : 8 * (i0 + c1 - c0)],
                num_idxs=128 * (c1 - c0), num_idxs_reg=regs["g"],
                elem_size=elem, single_packet=False, queue_num=qn)

        snap = None
        qn = issue_gat.next_q
        for cX, coff, ioff, tab in ((cA, 0, 0, tabA), (cB, cA, cA, tabB)):
            for c0 in range(0, cX, GCHUNK):
                c1 = min(c0 + GCHUNK, cX)
                snap = add("gpsimd",
                           lambda g, c0=c0 + coff, c1=c1 + coff,
                           i0=c0 + ioff, tab=tab, qn=qn:
                           _gather(g, c0, c1, i0, tab, qn),
                           waits=gw, inc=(f"g{qn}{bg}", 16))
                qn = (qn + 1) % 4
        issue_gat.next_q = qn
        return snap, cA + cB
    issue_gat.next_q = 0

    def attn_score(nm, c, bg, query_fn, first_waits, sidx,
                   extra_ap=None, lrelu=False):
        """Scores + softmax + paired exp; returns exp act count."""
        elem = gelem[nm]
        buf = gbuf[nm][bg]
        mask = mbuf[nm][bg]
        sc_, ep_, rd_ = s_sc[sidx], s_ep[sidx], s_red[sidx]
        if elem > 128:
            # alpha_src rides in the gathered row at column 128
            col = view_cf(buf[:, 0 : c * elem], c, elem)[
                :, :, 128:129].rearrange("p c f -> p (c f)")
            add("vector", lambda v, col=col, c=c, e=extra_ap:
                v.tensor_scalar_add(sc_[:, 0:c], col, e),
                waits=first_waits, inc=("dve", 1))
        else:
            q_ap = query_fn(c)
            add("vector", lambda v, c=c, q=q_ap, w1=s_w1[sidx]:
                v.tensor_tensor(
                    out=view_cf(w1[:, 0 : c * 128], c),
                    in0=view_cf(buf[:, 0 : c * 128], c), in1=q,
                    op=mb.AluOpType.mult),
                waits=first_waits, inc=("dve", 1))
            # packed bf16 add-tree (2x mode) halves the 1x TensorReduce work
            for wdt in (64, 32):
                add("vector", lambda v, c=c, wdt=wdt, w1=s_w1[sidx]:
                    v.tensor_tensor(
                        out=view_cf(w1[:, 0 : c * 128], c)[:, :, 0:wdt],
                        in0=view_cf(w1[:, 0 : c * 128], c)[:, :, 0:wdt],
                        in1=view_cf(w1[:, 0 : c * 128], c)[:, :, wdt:2 * wdt],
                        op=mb.AluOpType.add), inc=("dve", 1))
            add("vector", lambda v, c=c, w1=s_w1[sidx]: v.tensor_reduce(
                out=sc_[:, 0:c],
                in_=view_cf(w1[:, 0 : c * 128], c)[:, :, 0:32],
                axis=mb.AxisListType.X, op=mb.AluOpType.add),
                inc=("dve", 1))
        add("vector", lambda v, c=c, mask=mask: v.tensor_tensor(
            out=sc_[:, 0:c], in0=sc_[:, 0:c], in1=mask[:, 0:c],
            op=mb.AluOpType.add), inc=("dve", 1))
        if lrelu:
            lr_ = s_lr[sidx]
            add("vector", lambda v, c=c, lr_=lr_: v.tensor_scalar_mul(
                lr_[:, 0:c], sc_[:, 0:c], NEG_SLOPE), inc=("dve", 1))
            add("vector", lambda v, c=c, lr_=lr_: v.tensor_tensor(
                out=sc_[:, 0:c], in0=sc_[:, 0:c], in1=lr_[:, 0:c],
                op=mb.AluOpType.max), inc=("dve", 1))
        snap = add("vector", lambda v, c=c: v.tensor_reduce(
            out=rd_[:, 0:1], in_=sc_[:, 0:c], axis=mb.AxisListType.X,
            op=mb.AluOpType.max, negate=True), inc=("dve", 1))
        # paired exp on the scalar engine: out[p,c,2] bf16, accum = 2*denom
        snap = add("scalar", lambda sc, c=c: sc.activation(
            out=ep_view(s_ep[sidx], c),
            in_=sc_[:, 0:c].unsqueeze(2).broadcast_to([128, c, 2]),
            func=mb.ActivationFunctionType.Exp,
            bias=rd_[:, 0:1], accum_out=rd_[:, 1:2]),
            waits=[("dve", snap["dve"])], inc=("act", 1))
        return snap["act"]

    def attn_wsum(nm, c, bg, sidx, exp_act, mult_waits=(), pe_waits=()):
        """alpha-scale on DVE (2x packed), slab-sum on PE into p_acc.
        mult_waits guards s_w1[sidx] reuse (prior PE slab-sum done);
        pe_waits guards p_acc[sidx] reuse (prior ACT scale-copy done)."""
        elem = gelem[nm]
        buf = gbuf[nm][bg]
        rd_ = s_red[sidx]
        w1 = s_w1[sidx]
        add("vector", lambda v: v.reciprocal(rd_[:, 2:3], rd_[:, 1:2]),
            waits=[("act", exp_act)], inc=("dve", 1))
        if elem > 128:
            hv = (view_cf(buf[:, 0 : c * elem], c, elem)[:, :, 0:128]
                  .rearrange("p c (f two) -> p c f two", two=2))
        else:
            hv = view4(buf[:, 0 : c * 128], c)
        snap = add("vector", lambda v, c=c, hv=hv: v.tensor_tensor(
            out=view4(w1[:, 0 : c * 128], c), in0=hv,
            in1=ep_bcast(s_ep[sidx], c),
            op=mb.AluOpType.mult),
            waits=list(mult_waits), inc=("dve", 1))
        dv = snap["dve"]
        for k in range(c):
            snap = add("tensor", lambda te, k=k, c=c, w1=w1: te.matmul(
                p_acc[sidx][:, :], s_id2[:, :],
                w1[:, k * 128 : (k + 1) * 128],
                start=(k == 0), stop=(k == c - 1)),
                waits=([("dve", dv)] + list(pe_waits)) if k == 0 else (),
                inc=("pe", 1))
        return snap

    import os as _os
    _STOP = int(_os.environ.get("BUILD_STOP", "9"))
    NT1 = NTILE if _STOP >= 1 else 0

    # ---------------- phase 1: entity embedding with fused mm1
    # DVE stream is software-pipelined one tile deep: tile t's scores and
    # alpha-scaling issue first, then tile t-1's tail (s_o/mm1/s_h/write)
    # whose PE/ACT round-trips overlap tile t+1's score work.
    emb_dve_done, emb_gt_done = {}, {}
    mm_act, h_hw, act_done = {}, {}, {}
    wpe = {}    # (t, sidx) -> pe count after that family's slab-sum
    tr_pe = {}  # t -> pe count after that tile's transpose
    sc_act = {}  # t -> act count after that tile's scale-copies
    so_dve = {}  # t -> dve count after tile t's s_o add
    ent_sc_dve = {}  # t -> dve count after tile t's score+wsum section
    idx_snaps = {}
    tile_c = {}
    DEP = 5  # entity prefetch depth

    def ent_idx(tt):
        # idx DMAs issue DEP-1 tiles ahead of their gathers so the SP
        # block at each tile's h write cannot starve the gather prefetch;
        # the mask buffer's prior reader is the score mask-add of tt-DEP
        if not (0 <= tt < NT1):
            return
        rd = ent_sc_dve.get(tt - DEP, 0)
        rgt = emb_gt_done.get(tt - DEP, ())
        ixn = f"ix{tt % DEP}"
        issue_idx("h", fam_h, tt, tt % DEP, rd, rgt, ixn)
        idx_snaps[tt] = issue_idx("t", fam_t, tt, tt % DEP, rd, rgt, ixn)

    def ent_tail(t):
        if not (0 <= t < NT1):
            return
        b = t % 2
        bg = t % DEP
        hw = pt("hw", t)
        snap = add("vector", lambda v, b=b: v.tensor_tensor(
            out=s_o[b][:, :], in0=s_emb[0][:, :], in1=s_emb[1][:, :],
            op=mb.AluOpType.add),
            waits=[("act", sc_act[t]), ("pe", tr_pe.get(t - 2, 0))],
            inc=("dve", 1))
        so_dve[t] = snap["dve"]
        # fused mm1: x@W = (he+te)@W0 + relmm[r]  -> h_own
        snap = add("tensor", lambda te, b=b: te.transpose(
            out=p_tr[b][:, :], in_=s_o[b][:, :], identity=s_id1[:, :]),
            waits=[("dve", snap["dve"]), ("act", mm_act.get(t - 2, 0)),
                   ("w", W)],
            inc=("pe", 1))
        tr_pe[t] = snap["pe"]
        snap = add("scalar", lambda sc, b=b: sc.activation(
            out=s_xT[b][:, :], in_=p_tr[b][:, :],
            func=mb.ActivationFunctionType.Copy),
            waits=[("pe", snap["pe"])], inc=("act", 1))
        mm_act[t] = snap["act"]
        act_done[t] = snap["act"]
        snap = add("tensor", lambda te, b=b: te.matmul(
            p_mm[b][:, :], s_xT[b][:, :], s_waug1[:, :],
            start=True, stop=True),
            waits=[("act", snap["act"]),
                   ("dve", emb_dve_done.get(t - 2, 0))], inc=("pe", 1))
        rmm = s_rel[bg][:, 128 : 128 + HC]
        snap = add("vector", lambda v, b=b, rmm=rmm: v.tensor_tensor(
            out=s_h[b][:, 0:130], in0=p_mm[b][:, 0:130],
            in1=rmm[:, 0:130], op=mb.AluOpType.add),
            waits=[("pe", snap["pe"]), ("gp", GP_SH),
                   (hw, h_hw.get(t - 2, 0))],
            inc=("dve", 1))
        snap = add("vector", lambda v, b=b, t=t, rmm=rmm: v.tensor_tensor(
            out=s_ad[0][:, t : t + 1], in0=p_mm[b][:, 129:130],
            in1=rmm[:, 129:130], op=mb.AluOpType.add), inc=("dve", 1))
        emb_dve_done[t] = snap["dve"]
        # h write rides the sync queue so it never stalls the gather stream
        snap = add("sync", lambda s, t=t, b=b: s.dma_start(
            out=h_own[128 * t : 128 * (t + 1), :], in_=s_h[b][:, :]),
            waits=[("dve", snap["dve"])], inc=(hw, 16))
        h_hw[t] = snap[hw]
        # fence after each chunk's last tile: an SP-ordered ckf
        # increment signals the gpsimd collective the writes completed
        if t in CH_LAST:
            hwa_v, hwb_v = cnt["hwa"], cnt["hwb"]
            add("sync", lambda s: s.nop(),
                waits=[("hwa", hwa_v), ("hwb", hwb_v)], inc=("ckf", 1))

    for tt in range(DEP - 1):
        ent_idx(tt)
    for t in range(NT1):
        b = t % 2
        bg = t % DEP
        ixn = f"ix{bg}"
        reuse_g = emb_dve_done.get(t - DEP, 0)
        ent_idx(t + DEP - 1)

        def _relgather(g, t, bg):
            g.reg_mov(regs["g"], 128)
            return g.dma_gather(
                out_ap=s_rel[bg][:, :].unsqueeze(1),
                in_ap=d_rel[:, :], idxs_ap=s_ridx[:, 8 * t : 8 * t + 8],
                num_idxs=128, num_idxs_reg=regs["g"], elem_size=RELW)
        add("gpsimd", lambda g, t=t, bg=bg: _relgather(g, t, bg),
            waits=[("w", W), ("msa", MSA), ("msb", MSB),
                   ("pad", GP), ("dve", reuse_g)],
            inc=(f"g0{bg}", 16))
        ix_snap = idx_snaps[t]
        snap, ch = issue_gat(
            "h", fam_h, t, bg, l_attr[0:SPLIT, :], l_attr[WBOFF_A:N, :],
            ix_snap, reuse_g, [("msa", MSA), ("msb", MSB)], ixn)
        snap, ct = issue_gat(
            "t", fam_t, t, bg, l_attr[0:SPLIT, :], l_attr[WBOFF_A:N, :],
            ix_snap, reuse_g, [], ixn)
        tile_c[t] = (ch, ct)
        emb_gt_done[t] = gsnap(snap, bg)
        gw = list(gsnap(snap, bg))
        rp_fn = lambda cX, bg=bg: rep_mid(s_rel[bg][:, 0:128], cX)
        # both score stages first so each exp overlaps DVE work
        eh = attn_score("h", ch, bg, rp_fn,
                        gw + [("pe", wpe.get((t - 1, 0), 0))], 0)
        et = attn_score("t", ct, bg, rp_fn,
                        [("pe", wpe.get((t - 1, 1), 0))], 1)
        snap = attn_wsum("h", ch, bg, 0, eh,
                         pe_waits=[("act", sc_act.get(t - 1, 0))])
        wpe[(t, 0)] = snap["pe"]
        snap = attn_wsum("t", ct, bg, 1, et,
                         pe_waits=[("act", sc_act.get(t - 1, 0))])
        wpe[(t, 1)] = snap["pe"]
        ent_sc_dve[t] = cnt["dve"]
        # previous tile's tail (s_o add -> mm1 -> s_h -> h write + CC)
        ent_tail(t - 1)
        # PSUM -> SBUF with 1/denom scale on the scalar engine
        add("scalar", lambda sc: sc.activation(
            out=s_emb[0][:, :], in_=p_acc[0][:, :],
            func=mb.ActivationFunctionType.Copy, scale=s_red[0][:, 2:3]),
            waits=[("pe", wpe[(t, 0)]), ("dve", so_dve.get(t - 1, 0))],
            inc=("act", 1))
        snap = add("scalar", lambda sc: sc.activation(
            out=s_emb[1][:, :], in_=p_acc[1][:, :],
            func=mb.ActivationFunctionType.Copy, scale=s_red[1][:, 2:3]),
            waits=[("pe", wpe[(t, 1)])], inc=("act", 1))
        sc_act[t] = snap["act"]
        # chunk collectives: placed DEP tiles past their fence so the
        # gpsimd wait is satisfied when the gather stream reaches it
        if _STOP >= 3 and (t - DEP) in CH_LAST:
            _issue_cc_chunk(add, CH_LAST.index(t - DEP), h_own, d_h, l_h,
                            "ckf", "c1", "loc1")
    ent_tail(NT1 - 1)
    mm1 = dict(cnt)
    PE_ENT = cnt["pe"]

    if _STOP >= 3:
        for k in range(NCHUNK):
            if CH_LAST[k] + DEP >= NT1:
                _issue_cc_chunk(add, k, h_own, d_h, l_h,
                                "ckf", "c1", "loc1")
        LOC_H1 = cnt["loc1"]
    else:
        LOC_H1 = 0

    # ---------------- edge phases (layer 1 fuses mm2 -> h2_own)
    def edge_phase(layer, d_tab, out_dst, loc_sem, loc_need, fuse=False):
        # cross-phase initial guards: everything issued before this phase
        pe0, act0, dve0 = cnt["pe"], cnt["act"], cnt["dve"]
        gt0 = {r: gsnap(cnt, r) for r in range(3)}
        ed_done, ed_gt, tinfo = {}, {}, {}
        ed_ow = {}
        mm2_act = {-2: act0, -1: act0}
        h2_hw = {}
        ewpe = {-2: pe0, -1: pe0}   # g -> pe after slab-sum
        etr = {-2: pe0, -1: pe0}    # g -> pe after transpose
        eact_done = {-2: act0, -1: act0}
        bias = s_bias1 if layer == 1 else s_bias2
        ad_col = s_ad[layer - 1]

        def tail(g_i):
            b = g_i % 2
            ow, hw = pt("ow", g_i), pt("hw", g_i)
            c, exp_act = tinfo.pop(g_i)
            snap = attn_wsum(
                "e", c, g_i % 3, b, exp_act,
                mult_waits=[("pe", ewpe[g_i - 2])],
                pe_waits=[("act", eact_done[g_i - 2])])
            ewpe[g_i] = snap["pe"]
            guard = (("pe", etr[g_i - 2]) if fuse
                     else (pt("ow", g_i), ed_ow.get(g_i - 2, 0)))
            snap = add("scalar", lambda sc, b=b: sc.activation(
                out=(s_o[b] if fuse else s_of[b])[:, :],
                in_=p_acc[b][:, :],
                func=mb.ActivationFunctionType.Copy, scale=s_red[b][:, 2:3]),
                waits=[("pe", ewpe[g_i]), guard], inc=("act", 1))
            eact_done[g_i] = snap["act"]
            if fuse:
                snap = add("vector", lambda v, b=b, bias=bias:
                           v.tensor_tensor(
                               out=s_o[b][:, :], in0=s_o[b][:, :],
                               in1=bias[:, :], op=mb.AluOpType.add),
                           waits=[("act", snap["act"])], inc=("dve", 1))
                ed_done[g_i] = snap["dve"]
                # fused mm2: transpose s_o -> matmul waug2 -> h2_own
                snap = add("tensor", lambda te, b=b: te.transpose(
                    out=p_tr[b][:, :], in_=s_o[b][:, :],
                    identity=s_id1[:, :]),
                    waits=[("dve", snap["dve"]),
                           ("act", mm2_act.get(g_i - 2, 0)), ("w", W)],
                    inc=("pe", 1))
                etr[g_i] = snap["pe"]
                snap = add("scalar", lambda sc, b=b: sc.activation(
                    out=s_xT[b][:, :], in_=p_tr[b][:, :],
                    func=mb.ActivationFunctionType.Copy),
                    waits=[("pe", snap["pe"])], inc=("act", 1))
                snap = add("tensor", lambda te, b=b: te.matmul(
                    p_mm[b][:, :], s_xT[b][:, :], s_waug2[:, :],
                    start=True, stop=True),
                    waits=[("act", snap["act"]),
                           ("dve", ed_done.get(g_i - 2, dve0))],
                    inc=("pe", 1))
                snap = add("scalar", lambda sc, b=b: sc.activation(
                    out=s_h[b][:, 0:130], in_=p_mm[b][:, 0:130],
                    func=mb.ActivationFunctionType.Copy),
                    waits=[("pe", snap["pe"]), ("gp", GP_SH),
                           (hw, h2_hw.get(g_i - 2, 0))], inc=("act", 1))
                snap = add("scalar", lambda sc, b=b, g_i=g_i: sc.activation(
                    out=s_ad[1][:, g_i : g_i + 1], in_=p_mm[b][:, 129:130],
                    func=mb.ActivationFunctionType.Copy), inc=("act", 1))
                mm2_act[g_i] = snap["act"]
                eact_done[g_i] = snap["act"]
                snap = add("sync", lambda s, g_i=g_i, b=b: s.dma_start(
                    out=h2_own[128 * g_i : 128 * (g_i + 1), :],
                    in_=s_h[b][:, :]),
                    waits=[("act", snap["act"])], inc=(hw, 16))
                h2_hw[g_i] = snap[hw]
            else:
                snap = add("vector", lambda v, b=b, bias=bias:
                           v.tensor_tensor(
                               out=s_of[b][:, :], in0=s_of[b][:, :],
                               in1=bias[:, :], op=mb.AluOpType.add),
                           waits=[("act", snap["act"])],
                           inc=("dve", 1))
                ed_done[g_i] = snap["dve"]
                snap = add("sync", lambda s, g_i=g_i, b=b, od=out_dst:
                           s.dma_start(
                               out=od[128 * g_i : 128 * (g_i + 1), :],
                               in_=s_of[b][:, :]),
                           waits=[("dve", snap["dve"])], inc=(ow, 16))
                ed_ow[g_i] = snap[ow]

        ntl = NTILE if _STOP >= 4 else 0
        eidx_snaps, ed_sc_dve = {}, {}

        def edge_idx(gg):
            # mask buffer's prior reader is the score mask-add of gg-3
            if not (0 <= gg < ntl):
                return
            rd = ed_sc_dve.get(gg - 3, dve0)
            rgt = ed_gt.get(gg - 3, None)
            if rgt is None:
                rgt = gt0[gg % 3]
            ixn = f"ix{gg % 3}"
            eidx_snaps[gg] = issue_idx("e", fam_e, gg, gg % 3, rd, rgt, ixn)

        edge_idx(0)
        edge_idx(1)
        for g_i in range(ntl):
            b = g_i % 2
            bg = g_i % 3
            reuse_d = ed_done.get(g_i - 3, dve0)
            ixn = f"ix{bg}"
            edge_idx(g_i + 2)
            ix_snap = eidx_snaps[g_i]
            snap, c = issue_gat(
                "e", fam_e, g_i, bg, d_tab[0:SPLIT, :],
                d_tab[WBOFF_E:NTOT, :],
                ix_snap, reuse_d, [(loc_sem, loc_need)], ixn)
            ed_gt[g_i] = gsnap(snap, bg)
            gw = list(gsnap(snap, bg))
            exp_act = attn_score(
                "e", c, bg, None, gw, b,
                extra_ap=ad_col[:, g_i : g_i + 1], lrelu=True)
            ed_sc_dve[g_i] = cnt["dve"]
            tinfo[g_i] = (c, exp_act)
            if g_i >= 1:
                tail(g_i - 1)
        if ntl:
            tail(ntl - 1)
        return dict(cnt)

    edge_phase(1, l_h, None, "loc1", LOC_H1, fuse=True)
    if _STOP >= 5:
        # single core-major AllGather for h2, then a reordering localize
        # (core-major d_h2 -> chunk-major 256-stride l_h2, 7 strided DMAs)
        hwa_v, hwb_v = cnt["hwa"], cnt["hwb"]
        add("sync", lambda s: s.nop(),
            waits=[("hwa", hwa_v), ("hwb", hwb_v)], inc=("ckf2", 1))
        snap = add("gpsimd", lambda g: g.collective_compute(
            "AllGather", mb.AluOpType.bypass,
            replica_groups=[list(range(NCORE))],
            ins=[h2_own[:, :]], outs=[d_h2[:, :]]),
            waits=[("ckf2", 1)], inc=("c2a", 1))
        for k in range(NCHUNK):
            for ci in range(NCORE):
                r0 = CH_START[k] * 128
                rows = CH_ROWS[k]
                g0 = CH_GSTART[k]
                add("sync", lambda s, k=k, ci=ci, r0=r0, rows=rows, g0=g0:
                    s.dma_start(
                        out=l_h2[g0 + ci * rows : g0 + (ci + 1) * rows,
                                 0:HC],
                        in_=d_h2[ci * NPAD + r0 : ci * NPAD + r0 + rows,
                                 :]),
                    waits=[("c2a", snap["c2a"])], inc=("loc2", 16))
        LOC_H2 = cnt["loc2"]
        if _STOP >= 6:
            edge_phase(2, l_h2, out_ext, "loc2", LOC_H2)

    final = dict(cnt)
    import os
    if os.environ.get("BASS_PRINT_SEMS"):
        print("FINAL SEM COUNTS:", final)

    # ---------------- emit
    with ExitStack() as es:
        block = es.enter_context(nc.Block())
        sems = {s_name: es.enter_context(nc.semaphore(f"sem_{s_name}"))
                for s_name in SEMS}

        def make_body(eng_name):
            def body(eng):
                if eng_name == "gpsimd":
                    regs["g"] = es.enter_context(eng.register("gnum"))
                for emit, waits, inc in ops[eng_name]:
                    for s_name, val in waits:
                        eng.wait_ge(sems[s_name], val)
                    inst = emit(eng)
                    if inc is not None and inst is not None:
                        inst.then_inc(sems[inc[0]], inc[1])
                if eng_name == "gpsimd":
                    for s_name in SEMS:
                        if not s_name.startswith(("c1", "c2")) \
                                and final[s_name] > 0:
                            eng.wait_ge(sems[s_name], final[s_name])
            return body

        block.gpsimd(make_body("gpsimd"))
        block.sync(make_body("sync"))
        block.vector(make_body("vector"))
        block.scalar(make_body("scalar"))
        block.tensor(make_body("tensor"))

    nc.compile()
    st.close()
    return nc


def _issue_cc_chunk(add, k, own, d_full, l_full, ck_sem, cc_base, loc_sem):
    """AllGather chunk k (3-4 tiles) into its contiguous chunk-major block
    of d_full, then localize into 256-stride l_full (per-core 2-dim
    copies). Issued from gpsimd (NRT straight-line ordering); waits on
    the chunk's SP fence (engine-ordered ckf increment) so the gpsimd
    queue never stalls on later tiles; the blocking collective itself
    (~35-42us) hides under the 5-tile gather prefetch."""
    import concourse.mybir as mb
    r0 = CH_START[k] * 128
    rows = CH_ROWS[k]
    g0 = CH_GSTART[k]
    cc = cc_base + ("a" if k % 2 == 0 else "b")
    snap = add("gpsimd", lambda g: g.collective_compute(
        "AllGather", mb.AluOpType.bypass,
        replica_groups=[list(range(NCORE))],
        ins=[own[r0 : r0 + rows, :]],
        outs=[d_full[g0 : g0 + NCORE * rows, :]]),
        waits=[(ck_sem, k + 1)], inc=(cc, 1))
    for ci in range(NCORE):
        add("sync", lambda s, ci=ci: s.dma_start(
            out=l_full[g0 + ci * rows : g0 + (ci + 1) * rows, 0:HC],
            in_=d_full[g0 + ci * rows : g0 + (ci + 1) * rows, :]),
            waits=[(cc, snap[cc])], inc=(loc_sem, 16))


# ---------------------------------------------------------------- kernel() --


def _prepare(inputs):
    newpos = attr_perm(inputs["h_attributes"], inputs["t_attributes"])
    plan = make_plan(inputs["h_attributes"], inputs["t_attributes"],
                     inputs["r_idx"], inputs["edge_index"], newpos)
    weights = make_weights(
        inputs["attr_table"], inputs["rel_table"], inputs["femb_w"],
        inputs["femb_b"], inputs["gat1_w"], inputs["gat1_asrc"],
        inputs["gat1_adst"], inputs["gat1_b"], inputs["gat2_w"],
        inputs["gat2_asrc"], inputs["gat2_adst"], inputs["gat2_b"],
        newpos)
    in_maps = []
    for c in range(NCORE):
        m = dict(
            attr_tT=weights["attr_tT"], rel_tT=weights["rel_tT"],
            relmm=weights["relmm"], femb_wt=weights["femb_wt"],
            femb_b_row=weights["femb_b_row"],
            waug1=weights["waug1"], waug2=weights["waug2"],
            b1_rep=weights["b1_rep"], b2_rep=weights["b2_rep"],
            ident2=weights["ident2"], ident1=weights["ident1"],
            ridx=plan["ridx"][c],
        )
        for nm in ("h", "t", "e"):
            fam = plan[f"fam_{nm}"]
            m[f"{nm}_idx"] = fam["idx"][c]
            m[f"{nm}_mask"] = fam["mask"][c]
        in_maps.append(m)
    return plan, weights, in_maps


LAST_EXEC_NS = None


def kernel(**inputs):
    global LAST_EXEC_NS
    import os
    plan, weights, in_maps = _prepare(inputs)
    nc = build_bass(plan)
    from concourse.bass_utils import run_bass_kernel_spmd
    trace = bool(os.environ.get("BASS_KTRACE"))
    res = run_bass_kernel_spmd(nc, in_maps, list(range(NCORE)), trace=trace)
    if res.exec_time_ns is not None:
        LAST_EXEC_NS = res.exec_time_ns
    outs = np.stack([np.asarray(res.results[c]["out"]) for c in range(NCORE)])
    return np.ascontiguousarray(
        outs[plan["core_of"], plan["local_of"]].astype(np.float32))


def bench(n_iter=5, inputs=None):
    """Time repeated NEFF executions (inputs staged once; outputs donated
    fresh each iter). Returns (best_s, all_s)."""
    import time
    import jax
    from jax.sharding import Mesh, PartitionSpec, NamedSharding
    plan, weights, in_maps = _prepare(inputs)
    nc = build_bass(plan)
    from concourse import bass2jax

    import concourse.mybir as mybir
    bass2jax.install_neuronx_cc_hook()
    partition_name = (nc.partition_id_tensor.name
                      if nc.partition_id_tensor else None)
    in_names, out_names, out_avals, zero_outs = [], [], [], []
    for alloc in nc.m.functions[0].allocations:
        if not isinstance(alloc, mybir.MemoryLocationSet):
            continue
        name = alloc.memorylocations[0].name
        if alloc.kind == "ExternalInput":
            if name != partition_name:
                in_names.append(name)
        elif alloc.kind == "ExternalOutput":
            shape = tuple(alloc.tensor_shape)
            dtype = mybir.dt.np(alloc.dtype)
            out_names.append(name)
            out_avals.append(jax.core.ShapedArray(shape, dtype))
            zero_outs.append(np.zeros(shape, dtype))
    n_params = len(in_names)
    n_outs = len(out_avals)
    in_names.extend(out_names)
    if partition_name is not None:
        in_names.append(partition_name)
    donate = tuple(range(n_params, n_params + n_outs))

    def _body(*args):
        operands = list(args)
        if partition_name is not None:
            operands.append(bass2jax.partition_id_tensor())
        return tuple(bass2jax._bass_exec_p.bind(
            *operands, out_avals=tuple(out_avals), in_names=tuple(in_names),
            out_names=tuple(out_names), lowering_input_output_aliases=(),
            sim_require_finite=True, sim_require_nnan=True, nc=nc))

    from jax.experimental.shard_map import shard_map
    devices = jax.devices()[:NCORE]
    mesh = Mesh(np.asarray(devices), ("core",))
    in_specs = (PartitionSpec("core"),) * (n_params + n_outs)
    out_specs = (PartitionSpec("core"),) * len(out_names)
    fn = jax.jit(shard_map(_body, mesh=mesh, in_specs=in_specs,
                           out_specs=out_specs, check_rep=False),
                 donate_argnums=donate, keep_unused=True)
    if nc.dbg_addr is not None:
        in_maps = [{**m, nc.dbg_addr.name: np.zeros((1, 2), np.uint32)}
                   for m in in_maps]
    per_core = [[np.asarray(m[k]) for k in in_names[:n_params]]
                for m in in_maps]
    sh = NamedSharding(mesh, PartitionSpec("core"))
    concat_in = [jax.device_put(
        np.concatenate([per_core[c][i] for c in range(NCORE)], axis=0), sh)
        for i in range(n_params)]
    zglobal = [np.zeros((NCORE * z.shape[0], *z.shape[1:]), z.dtype)
               for z in zero_outs]
    times = []
    for it in range(n_iter):
        zs = [jax.device_put(z, sh) for z in zglobal]
        for z in zs:
            z.block_until_ready()
        t0 = time.perf_counter()
        outs = fn(*concat_in, *zs)
        for o in outs:
            o.block_until_ready()
        times.append(time.perf_counter() - t0)
    return min(times), times
